# revision 55
# baseline (speedup 1.0000x reference)
"""Trainium2 Bass kernel for the gene-network AE decoder (3 sparse layers).

Network (per reference):
  h1 = tanh(x @ A1 + b1)                A1: [1024, 80000], 16 nnz/col
  h2 = tanh(blockdiag4x4(W2) h1 + b2)   gene-local 4x4 mixing
  y  = blockdiag1x4(W3) h2 + b3         gene-local 4->1 reduction

Sharding: genes across the 8 cores (2500 genes -> padded to 2560 = 10240
nodes = 20 matmul tiles of 512). No inter-core communication: layer 1 only
needs the (replicated) 1024 TF features; layers 2/3 are gene-local.

The layer-1 sparse matrix is expanded to dense fp8 e3m4 on the host
(placement of the runtime w1 values at positions given by the runtime in1
indices; all arithmetic happens on device). e3m4 halves the dominant HBM
stream; w2/w3 stay bf16 (total quantization error ~1.5%, gate 2%).

Pipeline (per m-tile of 512 nodes = 128 genes), software-pipelined with
one-iteration slack between engine stages:

  t=tt:   L1   (PE)  8 chunk matmuls, xt stationary / a1 moving -> ps1
  t=tt+1: EVAC (ACT) ps1/8 -> s1 bf16;  T (PE, after L1(t)) -> psT [m,b];
          W2-expand (Pool) compact 16-col w2 -> 512-col block-diag
  t=tt+2: ADD1 (DVE) +b1;  TANH1 (ACT) -> h1T
  t=tt+3: L2   (PE)  4 block-diag W2 matmuls; ADD2 (DVE) +b2;
          TANH2 (ACT) -> h2T
  t=tt+4: L3   (PE)  4 W3 matmuls packed in one PSUM tile; ADD3 (DVE) +b3
          into a 2-tile output buffer
  t=tt+5: out DMA for each tile pair (odd tt)

PE is the binding engine (L1 37us + T 7 + L2 6 + L3 4 busy, ~96%
occupancy in span). Hard-won scheduling facts baked in here:
- each dma_start costs ~0.65us of sequencer issue time (DIRECT2D), so a1
  ships as 2-tile (1MB) DMAs from a flat [128, NT*4096] layout, w2/w3
  ride one fused 144-col stream (w2 ships compact and is expanded into
  its 97%-zeros block-diagonal form by the otherwise-idle Pool engine:
  mask x broadcast multiply), and outputs pair up 2 tiles per DMA;
- the sync HW queue starts ~2.4us before the scalar one, so xt leads the
  sync queue ahead of the a1 stream;
- a DMA whose semaphore wait isn't already satisfied blocks its queue's
  sequencer in-order, so the out DMA fires one iteration AFTER its data
  is complete, and the sync queue carries nothing but the a1 stream;
- the DMA XBAR transpose (dma_start_transpose) is NOT free - it occupies
  the issuing engine ~1.1us per [128,512] - so transposes stay on PE;
- fp8 e3m4 runs at 1 cycle/row (same as bf16) on PE; DoubleRow (0.5
  cyc/row) requires e4m3 whose 3 mantissa bits measure 2.8-4.0% rel err
  end-to-end - over the 2% gate, so no DoubleRow;
- a dummy tanh right after the const DMAs preloads the 1.3us ACT table
  during the DMA ramp;
- ptile q3 of the last tile is pure padding: its A/B chain is skipped,
  L1(last) runs 384-wide, and L3 uses a zeroed stationary stand-in.
Note: some runs execute at a 1.2x slower DVFS point (ACT_TABLE_LOAD
canary 1539ns vs nominal 1283ns); nominal-clock time is ~73-75us.
"""

import sys
import types

import numpy as np

try:
    import ml_dtypes
except ImportError:  # pragma: no cover
    ml_dtypes = None

import concourse.bass as bass
import concourse.tile as tile
from concourse import bacc, mybir
from concourse.bass_utils import run_bass_kernel_spmd

# ---------------------------------------------------------------- constants
B = 128          # batch
N_TF = 1024      # input features (= 8 chunks of 128)
N_GENES = 20000
W = 4            # nodes per gene
FANIN = 16
NCORES = 8
GC = N_GENES // NCORES      # 2500 genes / core
GP = 2560                   # padded genes / core
MP = GP * W                 # 10240 padded nodes / core
MT = 512                    # matmul moving tile (1 PSUM bank of f32)
NT = MP // MT               # 20 tiles / core
NCH = N_TF // 128           # 8 contraction chunks
A1SCALE = 8.0    # fp8e3 pre-scale: keeps w1 out of the e3m4 subnormal range
NPT = MP // 128             # 80 ptiles (128 nodes = 32 genes)

BF16 = mybir.dt.bfloat16
F32 = mybir.dt.float32
FP8 = mybir.dt.float8e3

_COMPILED = None


def _np_bf16():
    assert ml_dtypes is not None, "ml_dtypes required for bf16 host arrays"
    return ml_dtypes.bfloat16


def _np_fp8():
    assert ml_dtypes is not None, "ml_dtypes required for fp8 host arrays"
    return ml_dtypes.float8_e3m4


# ---------------------------------------------------------------- NTFF shim
def _install_ntff_shim():
    """Register the NTFF profile hook if this image's antenv lacks it."""
    try:
        import antenv
        if "antenv.axon_hooks" in sys.modules:
            return
        mod = types.ModuleType("antenv.axon_hooks")
        mod._hook = None
        mod.set_axon_ntff_profile_hook = lambda h: setattr(mod, "_hook", h)
        mod.get_axon_ntff_profile_hook = lambda: mod._hook
        sys.modules["antenv.axon_hooks"] = mod
        antenv.axon_hooks = mod
        from trn_agent_boot.trn_boot import _ntff_profile_via_ctypes
        mod.set_axon_ntff_profile_hook(
            _ntff_profile_via_ctypes("/opt/axon/libaxon_pjrt.so"))
    except Exception:
        pass


# ---------------------------------------------------------------- program
def _build_program():
    nc = bacc.Bacc("TRN2", target_bir_lowering=False, debug=False,
                   num_devices=NCORES)

    a1_ext = nc.dram_tensor("a1", [128, NT * NCH * MT], FP8,
                            kind="ExternalInput")
    xt_ext = nc.dram_tensor("xt", [128, N_TF], BF16, kind="ExternalInput")
    b1_ext = nc.dram_tensor("b1c", [128, NPT], BF16, kind="ExternalInput")
    w23_ext = nc.dram_tensor("w23m", [128, NT * 144], BF16,
                             kind="ExternalInput")
    mask_ext = nc.dram_tensor("maskb", [128, 128], BF16,
                              kind="ExternalInput")
    b2_ext = nc.dram_tensor("b2c", [128, NPT], F32, kind="ExternalInput")
    b3_ext = nc.dram_tensor("b3c", [128, NT], F32, kind="ExternalInput")
    id_ext = nc.dram_tensor("ident", [128, 128], BF16, kind="ExternalInput")
    # out[p, tt*128 + b] = y[b, gene tt*128 + p] (tile-major columns)
    out_ext = nc.dram_tensor("out", [128, NT * 128], BF16,
                             kind="ExternalOutput")

    LAST = NT - 1

    with tile.TileContext(nc) as tc:
        with (
            tc.tile_pool(name="consts", bufs=1) as consts,
            tc.tile_pool(name="a1p", bufs=4) as a1p,
            tc.tile_pool(name="w2sp", bufs=7) as w2sp,
            tc.tile_pool(name="w2xp", bufs=4) as w2xp,
            tc.tile_pool(name="ps1p", bufs=3, space="PSUM") as ps1p,
            tc.tile_pool(name="ps2p", bufs=1, space="PSUM") as ps2p,
            tc.tile_pool(name="ps3p", bufs=1, space="PSUM") as ps3p,
            tc.tile_pool(name="s1p", bufs=3) as s1p,
            tc.tile_pool(name="s1Tp", bufs=2, space="PSUM") as s1Tp,
            tc.tile_pool(name="s2p", bufs=3) as s2p,
            tc.tile_pool(name="s3p", bufs=3) as s3p,
            tc.tile_pool(name="h1Tp", bufs=3) as h1Tp,
            tc.tile_pool(name="h2Tp", bufs=3) as h2Tp,
            tc.tile_pool(name="outp", bufs=3) as outp,
        ):
            # the sync HW queue starts executing ~2.4us before the scalar
            # one, so xt leads the sync queue ahead of the a1 stream (both
            # gate the first matmul); consts ride the late scalar queue,
            # ident first (T(0) needs it soonest)
            xt = consts.tile([128, N_TF], BF16, tag="xt")
            nc.sync.dma_start(xt[:], xt_ext.ap())
            ident = consts.tile([128, 128], BF16, tag="ident")
            nc.scalar.dma_start(ident[:], id_ext.ap())
            b1c = consts.tile([128, NPT], BF16, tag="b1c")
            nc.scalar.dma_start(b1c[:], b1_ext.ap())
            b2c = consts.tile([128, NPT], F32, tag="b2c")
            nc.scalar.dma_start(b2c[:], b2_ext.ap())
            b3c = consts.tile([128, NT], F32, tag="b3c")
            nc.scalar.dma_start(b3c[:], b3_ext.ap())
            # 4x4 block-diagonal ones mask for the on-device w2 expansion
            maskb = consts.tile([128, 128], BF16, tag="maskb")
            nc.scalar.dma_start(maskb[:], mask_ext.ap())
            # stand-in for the all-padding ptile q3 of the last tile
            zero_h2 = consts.tile([128, 128], BF16, tag="zero_h2")
            nc.gpsimd.memset(zero_h2[:], 0)
            # preload the tanh ACT table during the DMA ramp so the first
            # real tanh doesn't eat the 1.3us table switch (reads xt: it
            # lands early on the sync queue)
            warm = consts.tile([128, 1], BF16, tag="warm")
            nc.scalar.activation(warm[:], xt[:, :1],
                                 mybir.ActivationFunctionType.Tanh)

            st = {}   # tile index -> dict of live tensors

            def stageW(tt):
                """expand compact w2 (16 cols) into the 512-col block-diag
                stationary on the idle Pool engine: 1 iter after its DMA."""
                d = st.setdefault(tt, {})
                w23t = d["w23t"]
                w2x = w2xp.tile([128, 512], BF16, tag="w2x",
                                name=f"w2x_{tt}")
                nc.gpsimd.tensor_tensor(
                    w2x[:].rearrange("p (q b i) -> p q b i", q=4, b=32),
                    maskb[:].rearrange("p (b i) -> p b i", b=32)[
                        :, None, :, :].to_broadcast([128, 4, 32, 4]),
                    w23t[:, :16].rearrange("p (q i) -> p q i", q=4)[
                        :, :, None, :].to_broadcast([128, 4, 32, 4]),
                    mybir.AluOpType.mult)
                d["w2x"] = w2x

            def stageA1e(tt, q0=0, q1=4):
                """evac (ACT): 1 iter after L1."""
                d = st[tt]
                nq = q1 - q0
                sfx = f"{tt}_{q0}"
                s1 = s1p.tile([128, nq * 128], BF16, tag="s1",
                              name=f"s1_{sfx}")
                nc.scalar.activation(s1[:], d["ps1"][:, q0 * 128:q1 * 128],
                                     mybir.ActivationFunctionType.Copy,
                                     scale=1.0 / A1SCALE)
                d[f"s1_{q0}"] = s1

            def stageA1t(tt, q0=0, q1=4):
                """transpose (PE): 1 iter after L1, placed after L1(t) in
                the PE stream so the evac has most of an iteration of
                slack."""
                d = st[tt]
                nq = q1 - q0
                sfx = f"{tt}_{q0}"
                s1 = d.pop(f"s1_{q0}")
                psT = s1Tp.tile([128, nq, 128], BF16, tag="psT",
                                name=f"psT_{sfx}")
                for q in range(nq):
                    nc.tensor.transpose(psT[:, q, :],
                                        s1[:, q * 128:(q + 1) * 128],
                                        ident[:])
                d[f"s1T_{q0}"] = psT

            def stageA2(tt, q0=0, q1=4):
                """+b1 (DVE, broadcast over batch) + tanh (ACT): 2 iters
                after L1."""
                d = st[tt]
                nq = q1 - q0
                sfx = f"{tt}_{q0}"
                s1T = d.pop(f"s1T_{q0}")
                s2 = s2p.tile([128, nq * 128], BF16, tag="s2",
                              name=f"s2_{sfx}")
                nc.vector.tensor_tensor(
                    s2[:].rearrange("p (q b) -> p q b", q=nq),
                    s1T[:],
                    b1c[:, tt * 4 + q0:tt * 4 + q1, None].to_broadcast(
                        [128, nq, 128]),
                    mybir.AluOpType.add)
                h1T = h1Tp.tile([128, nq * 128], BF16, tag="h1T",
                                name=f"h1T_{sfx}")
                nc.scalar.activation(h1T[:], s2[:],
                                     mybir.ActivationFunctionType.Tanh)
                d[f"h1T_{q0}"] = h1T

            def stageB(tt, q0=0, q1=4):
                """layer 2 (PE) + bias (DVE) + tanh (ACT): 3 iters after
                L1."""
                d = st[tt]
                nq = q1 - q0
                sfx = f"{tt}_{q0}"
                h1T = d.pop(f"h1T_{q0}")
                ps2 = ps2p.tile([128, nq * 128], F32, tag="ps2",
                                name=f"ps2_{sfx}")
                for q in range(nq):
                    nc.tensor.matmul(
                        ps2[:, q * 128:(q + 1) * 128],
                        d["w2x"][:, (q0 + q) * 128:(q0 + q + 1) * 128],
                        h1T[:, q * 128:(q + 1) * 128],
                        start=True, stop=True)
                s3 = s3p.tile([128, nq * 128], F32, tag="s3",
                              name=f"s3_{sfx}")
                nc.vector.tensor_tensor(
                    s3[:].rearrange("p (q b) -> p q b", q=nq),
                    ps2[:].rearrange("p (q b) -> p q b", q=nq),
                    b2c[:, tt * 4 + q0:tt * 4 + q1, None].to_broadcast(
                        [128, nq, 128]),
                    mybir.AluOpType.add)
                h2T = h2Tp.tile([128, nq * 128], BF16, tag="h2T",
                                name=f"h2T_{sfx}")
                nc.scalar.activation(h2T[:], s3[:],
                                     mybir.ActivationFunctionType.Tanh)
                d[f"h2T_{q0}"] = h2T

            def stageC(tt):
                """layer 3 (PE, full width) + b3 (DVE): 4 iters after L1."""
                d = st[tt]
                ps3 = ps3p.tile([128, 128], F32, tag="ps3",
                                name=f"ps3_{tt}")
                for q in range(4):
                    if tt == LAST:
                        # ptile q3 of the last tile is all padding
                        h2q = (zero_h2[:] if q == 3
                               else d.pop(f"h2T_{q}")[:, :128])
                    elif q == 0:
                        d["_h2T"] = d.pop("h2T_0")
                        h2q = d["_h2T"][:, :128]
                    else:
                        h2q = d["_h2T"][:, q * 128:(q + 1) * 128]
                    nc.tensor.matmul(
                        ps3[q * 32:(q + 1) * 32, :],
                        d["w3t"][:, q * 32:(q + 1) * 32],
                        h2q,
                        start=True, stop=True,
                        tile_position=(0, 32 * q))
                d.pop("_h2T", None)
                if tt % 2 == 0:
                    yt = outp.tile([128, 256], BF16, tag="yt",
                                   name=f"yt_{tt // 2}")
                    st[tt + 1]["ytbuf"] = yt
                else:
                    yt = d.pop("ytbuf")
                nc.vector.tensor_scalar_add(
                    yt[:, (tt % 2) * 128:(tt % 2 + 1) * 128], ps3[:],
                    b3c[:128, tt:tt + 1])
                d["yt"] = yt

            def stageD(tt):
                """out DMA for tile pair (tt-1, tt), odd tt: 5 iters after
                L1, one iteration after its yt half is written so the sem
                wait never blocks the queue sequencer."""
                if tt % 2 == 0:
                    return
                yt = st[tt].pop("yt")
                nc.scalar.dma_start(
                    out_ext.ap()[:, (tt - 1) * 128:(tt + 1) * 128], yt[:])

            def run(stage, tl, quarters=True):
                if 0 <= tl < NT:
                    if tl == LAST and quarters:
                        # q3 is all padding: skip its whole A/B chain
                        for q in range(3):
                            stage(tl, q, q + 1)
                    else:
                        stage(tl)

            for t in range(NT + 5):
                # ---- input DMAs (each dma_start costs ~0.65us of
                # sequencer issue time, so a1 rides in 2-tile chunks and
                # w2/w3 are fused into one 640-col stream)
                TW = NCH * MT   # 4096 a1 cols per m-tile
                if t < NT and t % 2 == 0:
                    a1t = a1p.tile([128, 2 * TW], FP8,
                                   tag="a1t", name=f"a1t_{t // 2}")
                    if t == 0:
                        # chunks 0-1 first so L1(0) starts after 128KB
                        nc.sync.dma_start(a1t[:, :2 * MT],
                                          a1_ext.ap()[:, :2 * MT])
                        nc.sync.dma_start(a1t[:, 2 * MT:TW],
                                          a1_ext.ap()[:, 2 * MT:TW])
                        nc.sync.dma_start(a1t[:, TW:],
                                          a1_ext.ap()[:, TW:2 * TW])
                    else:
                        nc.sync.dma_start(
                            a1t[:], a1_ext.ap()[:, t * TW:(t + 2) * TW])
                    st.setdefault(t, {})["a1t"] = a1t
                    st.setdefault(t + 1, {})["a1t"] = a1t
                if t < NT:
                    # compact stream: 16 w2 cols (4x4 per gene-node) + 128
                    # dense w3 cols; w2 is expanded on the idle Pool engine
                    w23t = w2sp.tile([128, 144], BF16, tag="w23t",
                                     name=f"w23t_{t}")
                    nc.scalar.dma_start(
                        w23t[:], w23_ext.ap()[:, t * 144:(t + 1) * 144])

                # engine-stream order per iteration:
                #   ACT: evac(t-1), tanh1(t-2), tanh2(t-3)
                #   DVE: add1(t-2), add2(t-3), add3(t-4)
                #   PE:  L1(t) first (ps1 completes early: feeds evac next
                #        iter and pulls the last tile's drain chain in),
                #        then L2(t-3), T(t-1), L3(t-4)
                run(stageA1e, t - 1)
                run(stageW, t - 1, quarters=False)
                run(stageA2, t - 2)

                if t < NT:
                    ps1 = ps1p.tile([128, MT], F32, tag="ps1",
                                    name=f"ps1_{t}")
                    a1v = st[t]
                    a1v["ps1"] = ps1
                    a1v["w23t"] = w23t[:]
                    a1v["w3t"] = w23t[:, 16:144]
                    a1m = a1v.pop("a1t")
                    off = (t % 2) * TW
                    mw = 384 if t == LAST else MT  # last ptile is padding
                    for ch in range(NCH):
                        nc.tensor.matmul(
                            ps1[:, :mw],
                            xt[:, ch * 128:(ch + 1) * 128],
                            a1m[:, off + ch * MT:off + ch * MT + mw],
                            start=(ch == 0), stop=(ch == NCH - 1))

                run(stageB, t - 3)
                run(stageA1t, t - 1)
                run(stageC, t - 4, quarters=False)
                run(stageD, t - 5, quarters=False)

    nc.compile()
    return nc


# ---------------------------------------------------------------- host prep
def _prep_core(c, w1, b1, w2, b2, w3, b3, in1):
    """Build the per-core input arrays (index/layout placement only)."""
    bf16 = _np_bf16()
    fp8 = _np_fp8()
    MC = GC * W  # 10000 real nodes per core

    # --- layer-1 dense matrix [1024, MP], columns = local node id 4g+j
    m_glob0 = (GC * c) * W
    e_idx = m_glob0 * FANIN + np.arange(MC * FANIN)
    t = in1[e_idx].astype(np.int64)                 # [MC*16]
    wv = w1[e_idx].astype(np.float64)
    mloc = np.repeat(np.arange(MC, dtype=np.int64), FANIN)
    A1 = np.bincount(t * MP + mloc, weights=wv,
                     minlength=N_TF * MP).reshape(N_TF, MP)
    # flat layout: a1[p, tt*4096 + ch*512 + j] = A1s[ch*128+p, tt*512+j]
    a1_packed = ((A1 * A1SCALE).reshape(NCH, 128, NT, MT)
                 .transpose(1, 2, 0, 3)
                 .reshape(128, NT * NCH * MT)
                 .astype(np.float32).astype(fp8))

    b1p = np.zeros(MP, np.float32)
    b1p[:MC] = b1[m_glob0:m_glob0 + MC]
    b1c = np.ascontiguousarray(b1p.reshape(NPT, 128).T).astype(bf16)

    # --- padded per-gene weights
    w2pad = np.zeros((GP, W, W), np.float32)        # [gene, i, j]
    w2pad[:GC] = w2.reshape(N_GENES, W, W)[GC * c:GC * (c + 1)]
    b2pad = np.zeros((GP, W), np.float32)
    b2pad[:GC] = b2.reshape(N_GENES, W)[GC * c:GC * (c + 1)]
    w3pad = np.zeros((GP, W), np.float32)
    w3pad[:GC] = w3.reshape(N_GENES, W)[GC * c:GC * (c + 1)]
    b3pad = np.zeros(GP, np.float32)
    b3pad[:GC] = b3[GC * c:GC * (c + 1)]

    # --- compact W2: w2c[tt, p=(a,j), q*4+i] = w2[(tt*4+q)*32+a, i, j];
    # the kernel expands it to the block-diag stationary on device
    w2c = (w2pad.reshape(NT, 4, 32, W, W)
           .transpose(0, 2, 4, 1, 3)       # [tt, a, j, q, i]
           .reshape(NT, 128, 16))

    # --- W3 mats: W3m[pt, (a,i), b] = d(a==b) w3[g*4+i]
    idx = np.arange(32)
    W3m = np.zeros((NPT, 32, W, 32), np.float32)
    W3m[:, idx, :, idx] = w3pad.reshape(NPT, 32, W).transpose(1, 0, 2)
    w3m = W3m.reshape(NT, 4, 128, 32)   # [tt, q, (a,j), b]

    # fused per-tile weight stream: 16 compact w2 cols + 128 w3 cols
    w23 = np.zeros((NT, 128, 144), np.float32)
    w23[:, :, :16] = w2c
    w23[:, :, 16:] = w3m.transpose(0, 2, 1, 3).reshape(NT, 128, 128)
    w23m = np.ascontiguousarray(
        w23.transpose(1, 0, 2).reshape(128, NT * 144)).astype(bf16)

    # --- bias columns
    b2c = (b2pad.reshape(NPT, 32, W).transpose(1, 2, 0)
           .reshape(128, NPT).astype(np.float32))
    b3c = np.ascontiguousarray(b3pad.reshape(NT, 128).T)

    return {
        "a1": a1_packed,
        "b1c": b1c,
        "w23m": w23m,
        "b2c": b2c,
        "b3c": b3c,
    }


def _run(inputs, trace=False):
    global _COMPILED
    if _COMPILED is None:
        _COMPILED = _build_program()
    nc = _COMPILED

    bf16 = _np_bf16()
    features = np.asarray(inputs["features"], np.float32)
    w1 = np.asarray(inputs["w1"], np.float32)
    b1 = np.asarray(inputs["b1"], np.float32)
    w2 = np.asarray(inputs["w2"], np.float32)
    b2 = np.asarray(inputs["b2"], np.float32)
    w3 = np.asarray(inputs["w3"], np.float32)
    b3 = np.asarray(inputs["b3"], np.float32)
    in1 = np.asarray(inputs["in1"], np.int32)

    # stationary x: [p, ch*128 + b] = x[b, ch*128 + p]
    xt = (features.T.reshape(NCH, 128, B).transpose(1, 0, 2)
          .reshape(128, N_TF).astype(bf16))
    ident = np.eye(128, dtype=np.float32).astype(bf16)
    maskb = np.kron(np.eye(32, dtype=np.float32),
                    np.ones((4, 4), np.float32)).astype(bf16)

    in_maps = []
    for c in range(NCORES):
        m = _prep_core(c, w1, b1, w2, b2, w3, b3, in1)
        m["xt"] = xt
        m["ident"] = ident
        m["maskb"] = maskb
        in_maps.append(m)

    if trace:
        _install_ntff_shim()
    res = run_bass_kernel_spmd(nc, in_maps, core_ids=list(range(NCORES)),
                               trace=trace)
    y = np.empty((B, N_GENES), np.float32)
    for c in range(NCORES):
        yc = np.asarray(res.results[c]["out"]).astype(np.float32)  # [128,NT*128]
        # yc[p, tt*128 + b] = y[b, GC*c + tt*128 + p]
        yg = yc.reshape(128, NT, 128).transpose(2, 1, 0).reshape(B, GP)
        y[:, GC * c:GC * (c + 1)] = yg[:, :GC]
    return y, res.exec_time_ns


def kernel(**inputs) -> np.ndarray:
    y, _ = _run(inputs, trace=False)
    return y


# revision 57
# speedup vs baseline: 1.0110x; 1.0110x over previous
"""Trainium2 Bass kernel for the gene-network AE decoder (3 sparse layers).

Network (per reference):
  h1 = tanh(x @ A1 + b1)                A1: [1024, 80000], 16 nnz/col
  h2 = tanh(blockdiag4x4(W2) h1 + b2)   gene-local 4x4 mixing
  y  = blockdiag1x4(W3) h2 + b3         gene-local 4->1 reduction

Sharding: genes across the 8 cores (2500 genes -> padded to 2560 = 10240
nodes = 20 matmul tiles of 512). No inter-core communication: layer 1 only
needs the (replicated) 1024 TF features; layers 2/3 are gene-local.

The layer-1 sparse matrix is expanded to dense fp8 e3m4 on the host
(placement of the runtime w1 values at positions given by the runtime in1
indices; all arithmetic happens on device). e3m4 halves the dominant HBM
stream; w2/w3 stay bf16 (total quantization error ~1.5%, gate 2%).

Pipeline (per m-tile of 512 nodes = 128 genes), software-pipelined with
one-iteration slack between engine stages:

  t=tt:   L1   (PE)  8 chunk matmuls, xt stationary / a1 moving -> ps1
  t=tt+1: EVAC (ACT) ps1/8 -> s1 bf16;  T (PE, after L1(t)) -> psT [m,b];
          W2-expand (Pool) compact 16-col w2 -> 512-col block-diag
  t=tt+2: ADD1 (DVE) +b1;  TANH1 (ACT) -> h1T
  t=tt+3: L2   (PE)  4 block-diag W2 matmuls; ADD2 (DVE) +b2;
          TANH2 (ACT) -> h2T
  t=tt+4: L3   (PE)  4 W3 matmuls packed in one PSUM tile; ADD3 (DVE) +b3
          into a 2-tile output buffer
  t=tt+5: out DMA for each tile pair (odd tt)

PE is the binding engine (L1 37us + T 7 + L2 6 + L3 4 busy, ~96%
occupancy in span). Hard-won scheduling facts baked in here:
- each dma_start costs ~0.65us of sequencer issue time (DIRECT2D), so a1
  ships as 2-tile (1MB) DMAs from a flat [128, NT*4096] layout, w2/w3
  ride one fused 144-col stream (w2 ships compact and is expanded into
  its 97%-zeros block-diagonal form by the otherwise-idle Pool engine:
  mask x broadcast multiply), and outputs pair up 2 tiles per DMA;
- the sync HW queue starts ~2.4us before the scalar one, so xt leads the
  sync queue ahead of the a1 stream;
- a DMA whose semaphore wait isn't already satisfied blocks its queue's
  sequencer in-order, so the out DMA fires one iteration AFTER its data
  is complete, and the sync queue carries nothing but the a1 stream;
- the DMA XBAR transpose (dma_start_transpose) is NOT free - it occupies
  the issuing engine ~1.1us per [128,512] - so transposes stay on PE;
- fp8 e3m4 runs at 1 cycle/row (same as bf16) on PE; DoubleRow (0.5
  cyc/row) requires e4m3 whose 3 mantissa bits measure 2.8-4.0% rel err
  end-to-end - over the 2% gate, so no DoubleRow;
- a dummy tanh right after the const DMAs preloads the 1.3us ACT table
  during the DMA ramp;
- ptile q3 of the last tile is pure padding: its A/B chain is skipped,
  L1(last) runs 384-wide, and L3 uses a zeroed stationary stand-in.
Note: some runs execute at a 1.2x slower DVFS point (ACT_TABLE_LOAD
canary 1539ns vs nominal 1283ns); nominal-clock time is ~73-75us.
"""

import sys
import types

import numpy as np

try:
    import ml_dtypes
except ImportError:  # pragma: no cover
    ml_dtypes = None

import concourse.bass as bass
import concourse.tile as tile
from concourse import bacc, mybir
from concourse.bass_utils import run_bass_kernel_spmd

# ---------------------------------------------------------------- constants
B = 128          # batch
N_TF = 1024      # input features (= 8 chunks of 128)
N_GENES = 20000
W = 4            # nodes per gene
FANIN = 16
NCORES = 8
GC = N_GENES // NCORES      # 2500 genes / core
GP = 2560                   # padded genes / core
MP = GP * W                 # 10240 padded nodes / core
MT = 512                    # matmul moving tile (1 PSUM bank of f32)
NT = MP // MT               # 20 tiles / core
NCH = N_TF // 128           # 8 contraction chunks
A1SCALE = 8.0    # fp8e3 pre-scale: keeps w1 out of the e3m4 subnormal range
NPT = MP // 128             # 80 ptiles (128 nodes = 32 genes)

BF16 = mybir.dt.bfloat16
F32 = mybir.dt.float32
FP8 = mybir.dt.float8e3

_COMPILED = None


def _np_bf16():
    assert ml_dtypes is not None, "ml_dtypes required for bf16 host arrays"
    return ml_dtypes.bfloat16


def _np_fp8():
    assert ml_dtypes is not None, "ml_dtypes required for fp8 host arrays"
    return ml_dtypes.float8_e3m4


# ---------------------------------------------------------------- NTFF shim
def _install_ntff_shim():
    """Register the NTFF profile hook if this image's antenv lacks it."""
    try:
        import antenv
        if "antenv.axon_hooks" in sys.modules:
            return
        mod = types.ModuleType("antenv.axon_hooks")
        mod._hook = None
        mod.set_axon_ntff_profile_hook = lambda h: setattr(mod, "_hook", h)
        mod.get_axon_ntff_profile_hook = lambda: mod._hook
        sys.modules["antenv.axon_hooks"] = mod
        antenv.axon_hooks = mod
        from trn_agent_boot.trn_boot import _ntff_profile_via_ctypes
        mod.set_axon_ntff_profile_hook(
            _ntff_profile_via_ctypes("/opt/axon/libaxon_pjrt.so"))
    except Exception:
        pass


# ---------------------------------------------------------------- program
def _build_program():
    nc = bacc.Bacc("TRN2", target_bir_lowering=False, debug=False,
                   num_devices=NCORES)

    a1_ext = nc.dram_tensor("a1", [128, NT * NCH * MT], FP8,
                            kind="ExternalInput")
    xt_ext = nc.dram_tensor("xt", [128, N_TF], BF16, kind="ExternalInput")
    b1_ext = nc.dram_tensor("b1c", [128, NPT], BF16, kind="ExternalInput")
    w23_ext = nc.dram_tensor("w23m", [128, NT * 144], BF16,
                             kind="ExternalInput")
    mask_ext = nc.dram_tensor("maskb", [128, 128], BF16,
                              kind="ExternalInput")
    b2_ext = nc.dram_tensor("b2c", [128, NPT], F32, kind="ExternalInput")
    b3_ext = nc.dram_tensor("b3c", [128, NT], F32, kind="ExternalInput")
    id_ext = nc.dram_tensor("ident", [128, 128], BF16, kind="ExternalInput")
    # out[p, tt*128 + b] = y[b, gene tt*128 + p] (tile-major columns)
    out_ext = nc.dram_tensor("out", [128, NT * 128], BF16,
                             kind="ExternalOutput")

    LAST = NT - 1

    with tile.TileContext(nc) as tc:
        with (
            tc.tile_pool(name="consts", bufs=1) as consts,
            tc.tile_pool(name="a1p", bufs=4) as a1p,
            tc.tile_pool(name="w2sp", bufs=7) as w2sp,
            tc.tile_pool(name="w2xp", bufs=4) as w2xp,
            tc.tile_pool(name="ps1p", bufs=3, space="PSUM") as ps1p,
            tc.tile_pool(name="ps2p", bufs=1, space="PSUM") as ps2p,
            tc.tile_pool(name="ps3p", bufs=1, space="PSUM") as ps3p,
            tc.tile_pool(name="s1p", bufs=3) as s1p,
            tc.tile_pool(name="s1Tp", bufs=2, space="PSUM") as s1Tp,
            tc.tile_pool(name="s2p", bufs=3) as s2p,
            tc.tile_pool(name="s3p", bufs=3) as s3p,
            tc.tile_pool(name="h1Tp", bufs=3) as h1Tp,
            tc.tile_pool(name="h2Tp", bufs=3) as h2Tp,
            tc.tile_pool(name="outp", bufs=3) as outp,
        ):
            # the sync HW queue starts executing ~2.4us before the scalar
            # one, so xt leads the sync queue ahead of the a1 stream (both
            # gate the first matmul); consts ride the late scalar queue,
            # ident first (T(0) needs it soonest)
            xt = consts.tile([128, N_TF], BF16, tag="xt")
            nc.sync.dma_start(xt[:], xt_ext.ap())
            ident = consts.tile([128, 128], BF16, tag="ident")
            nc.scalar.dma_start(ident[:], id_ext.ap())
            b1c = consts.tile([128, NPT], BF16, tag="b1c")
            nc.scalar.dma_start(b1c[:], b1_ext.ap())
            b2c = consts.tile([128, NPT], F32, tag="b2c")
            nc.scalar.dma_start(b2c[:], b2_ext.ap())
            b3c = consts.tile([128, NT], F32, tag="b3c")
            nc.scalar.dma_start(b3c[:], b3_ext.ap())
            # 4x4 block-diagonal ones mask for the on-device w2 expansion
            maskb = consts.tile([128, 128], BF16, tag="maskb")
            nc.scalar.dma_start(maskb[:], mask_ext.ap())
            # stand-in for the all-padding ptile q3 of the last tile
            zero_h2 = consts.tile([128, 128], BF16, tag="zero_h2")
            nc.gpsimd.memset(zero_h2[:], 0)
            # preload the tanh ACT table during the DMA ramp so the first
            # real tanh doesn't eat the 1.3us table switch (reads xt: it
            # lands early on the sync queue)
            warm = consts.tile([128, 1], BF16, tag="warm")
            nc.scalar.activation(warm[:], xt[:, :1],
                                 mybir.ActivationFunctionType.Tanh)

            st = {}   # tile index -> dict of live tensors

            def stageW(tt):
                """expand compact w2 (16 cols) into the 512-col block-diag
                stationary on the idle Pool engine: 1 iter after its DMA."""
                d = st.setdefault(tt, {})
                w23t = d["w23t"]
                w2x = w2xp.tile([128, 512], BF16, tag="w2x",
                                name=f"w2x_{tt}")
                nc.gpsimd.tensor_tensor(
                    w2x[:].rearrange("p (q b i) -> p q b i", q=4, b=32),
                    maskb[:].rearrange("p (b i) -> p b i", b=32)[
                        :, None, :, :].to_broadcast([128, 4, 32, 4]),
                    w23t[:, :16].rearrange("p (q i) -> p q i", q=4)[
                        :, :, None, :].to_broadcast([128, 4, 32, 4]),
                    mybir.AluOpType.mult)
                d["w2x"] = w2x

            def stageA1e(tt, q0=0, q1=4):
                """evac (ACT): 1 iter after L1."""
                d = st[tt]
                nq = q1 - q0
                sfx = f"{tt}_{q0}"
                s1 = s1p.tile([128, nq * 128], BF16, tag="s1",
                              name=f"s1_{sfx}")
                nc.scalar.activation(s1[:], d["ps1"][:, q0 * 128:q1 * 128],
                                     mybir.ActivationFunctionType.Copy,
                                     scale=1.0 / A1SCALE)
                d[f"s1_{q0}"] = s1

            def stageA1t(tt, q0=0, q1=4):
                """transpose (PE): 1 iter after L1, placed after L1(t) in
                the PE stream so the evac has most of an iteration of
                slack."""
                d = st[tt]
                nq = q1 - q0
                sfx = f"{tt}_{q0}"
                s1 = d.pop(f"s1_{q0}")
                psT = s1Tp.tile([128, nq, 128], BF16, tag="psT",
                                name=f"psT_{sfx}")
                for q in range(nq):
                    nc.tensor.transpose(psT[:, q, :],
                                        s1[:, q * 128:(q + 1) * 128],
                                        ident[:])
                d[f"s1T_{q0}"] = psT

            def stageA2(tt, q0=0, q1=4):
                """+b1 (DVE, broadcast over batch) + tanh (ACT): 2 iters
                after L1."""
                d = st[tt]
                nq = q1 - q0
                sfx = f"{tt}_{q0}"
                s1T = d.pop(f"s1T_{q0}")
                s2 = s2p.tile([128, nq * 128], BF16, tag="s2",
                              name=f"s2_{sfx}")
                nc.vector.tensor_tensor(
                    s2[:].rearrange("p (q b) -> p q b", q=nq),
                    s1T[:],
                    b1c[:, tt * 4 + q0:tt * 4 + q1, None].to_broadcast(
                        [128, nq, 128]),
                    mybir.AluOpType.add)
                h1T = h1Tp.tile([128, nq * 128], BF16, tag="h1T",
                                name=f"h1T_{sfx}")
                nc.scalar.activation(h1T[:], s2[:],
                                     mybir.ActivationFunctionType.Tanh)
                d[f"h1T_{q0}"] = h1T

            def stageB(tt, q0=0, q1=4):
                """layer 2 (PE) + bias (DVE) + tanh (ACT): 3 iters after
                L1."""
                d = st[tt]
                nq = q1 - q0
                sfx = f"{tt}_{q0}"
                h1T = d.pop(f"h1T_{q0}")
                ps2 = ps2p.tile([128, nq * 128], F32, tag="ps2",
                                name=f"ps2_{sfx}")
                for q in range(nq):
                    nc.tensor.matmul(
                        ps2[:, q * 128:(q + 1) * 128],
                        d["w2x"][:, (q0 + q) * 128:(q0 + q + 1) * 128],
                        h1T[:, q * 128:(q + 1) * 128],
                        start=True, stop=True)
                s3 = s3p.tile([128, nq * 128], F32, tag="s3",
                              name=f"s3_{sfx}")
                nc.vector.tensor_tensor(
                    s3[:].rearrange("p (q b) -> p q b", q=nq),
                    ps2[:].rearrange("p (q b) -> p q b", q=nq),
                    b2c[:, tt * 4 + q0:tt * 4 + q1, None].to_broadcast(
                        [128, nq, 128]),
                    mybir.AluOpType.add)
                h2T = h2Tp.tile([128, nq * 128], BF16, tag="h2T",
                                name=f"h2T_{sfx}")
                nc.scalar.activation(h2T[:], s3[:],
                                     mybir.ActivationFunctionType.Tanh)
                d[f"h2T_{q0}"] = h2T

            def stageC(tt):
                """layer 3 (PE, full width) + b3 (DVE): 4 iters after L1."""
                d = st[tt]
                ps3 = ps3p.tile([128, 128], F32, tag="ps3",
                                name=f"ps3_{tt}")
                for q in range(4):
                    if tt == LAST:
                        # ptile q3 of the last tile is all padding
                        h2q = (zero_h2[:] if q == 3
                               else d.pop(f"h2T_{q}")[:, :128])
                    elif q == 0:
                        d["_h2T"] = d.pop("h2T_0")
                        h2q = d["_h2T"][:, :128]
                    else:
                        h2q = d["_h2T"][:, q * 128:(q + 1) * 128]
                    nc.tensor.matmul(
                        ps3[q * 32:(q + 1) * 32, :],
                        d["w3t"][:, q * 32:(q + 1) * 32],
                        h2q,
                        start=True, stop=True,
                        tile_position=(0, 32 * q))
                d.pop("_h2T", None)
                if tt % 2 == 0:
                    yt = outp.tile([128, 256], BF16, tag="yt",
                                   name=f"yt_{tt // 2}")
                    st[tt + 1]["ytbuf"] = yt
                else:
                    yt = d.pop("ytbuf")
                nc.vector.tensor_scalar_add(
                    yt[:, (tt % 2) * 128:(tt % 2 + 1) * 128], ps3[:],
                    b3c[:128, tt:tt + 1])
                d["yt"] = yt

            def stageD(tt):
                """out DMA for tile pair (tt-1, tt), odd tt: 5 iters after
                L1, one iteration after its yt half is written so the sem
                wait never blocks the queue sequencer."""
                if tt % 2 == 0:
                    return
                yt = st[tt].pop("yt")
                nc.scalar.dma_start(
                    out_ext.ap()[:, (tt - 1) * 128:(tt + 1) * 128], yt[:])

            def run(stage, tl, quarters=True):
                if 0 <= tl < NT:
                    if tl == LAST and quarters:
                        # q3 is all padding: skip its whole A/B chain
                        for q in range(3):
                            stage(tl, q, q + 1)
                    else:
                        stage(tl)

            for t in range(NT + 5):
                # ---- input DMAs (each dma_start costs ~0.65us of
                # sequencer issue time, so a1 rides in 2-tile chunks and
                # w2/w3 are fused into one 640-col stream)
                TW = NCH * MT   # 4096 a1 cols per m-tile
                if t < NT and t % 2 == 0:
                    a1t = a1p.tile([128, 2 * TW], FP8,
                                   tag="a1t", name=f"a1t_{t // 2}")
                    if t == 0:
                        # chunks 0-1 first so L1(0) starts after 128KB
                        nc.sync.dma_start(a1t[:, :2 * MT],
                                          a1_ext.ap()[:, :2 * MT])
                        nc.sync.dma_start(a1t[:, 2 * MT:TW],
                                          a1_ext.ap()[:, 2 * MT:TW])
                        nc.sync.dma_start(a1t[:, TW:],
                                          a1_ext.ap()[:, TW:2 * TW])
                    else:
                        nc.sync.dma_start(
                            a1t[:], a1_ext.ap()[:, t * TW:(t + 2) * TW])
                    st.setdefault(t, {})["a1t"] = a1t
                    st.setdefault(t + 1, {})["a1t"] = a1t
                if t < NT:
                    # compact stream: 16 w2 cols (4x4 per gene-node) + 128
                    # dense w3 cols; w2 is expanded on the idle Pool engine
                    w23t = w2sp.tile([128, 144], BF16, tag="w23t",
                                     name=f"w23t_{t}")
                    nc.scalar.dma_start(
                        w23t[:], w23_ext.ap()[:, t * 144:(t + 1) * 144])

                # engine-stream order per iteration:
                #   ACT: evac(t-1), tanh1(t-2), tanh2(t-3)
                #   DVE: add1(t-2), add2(t-3), add3(t-4)
                #   PE:  L2(t-3), L1(t), T(t-1), L3(t-4) — L2 before L1 so
                #        PE does ready work while a1(t) may still be in
                #        flight (L1-first measured 2us slower)
                run(stageA1e, t - 1)
                run(stageW, t - 1, quarters=False)
                run(stageA2, t - 2)
                run(stageB, t - 3)

                if t < NT:
                    ps1 = ps1p.tile([128, MT], F32, tag="ps1",
                                    name=f"ps1_{t}")
                    a1v = st[t]
                    a1v["ps1"] = ps1
                    a1v["w23t"] = w23t[:]
                    a1v["w3t"] = w23t[:, 16:144]
                    a1m = a1v.pop("a1t")
                    off = (t % 2) * TW
                    mw = 384 if t == LAST else MT  # last ptile is padding
                    for ch in range(NCH):
                        nc.tensor.matmul(
                            ps1[:, :mw],
                            xt[:, ch * 128:(ch + 1) * 128],
                            a1m[:, off + ch * MT:off + ch * MT + mw],
                            start=(ch == 0), stop=(ch == NCH - 1))

                run(stageA1t, t - 1)
                run(stageC, t - 4, quarters=False)
                run(stageD, t - 5, quarters=False)

    nc.compile()
    return nc


# ---------------------------------------------------------------- host prep
def _prep_core(c, w1, b1, w2, b2, w3, b3, in1):
    """Build the per-core input arrays (index/layout placement only)."""
    bf16 = _np_bf16()
    fp8 = _np_fp8()
    MC = GC * W  # 10000 real nodes per core

    # --- layer-1 dense matrix [1024, MP], columns = local node id 4g+j
    m_glob0 = (GC * c) * W
    e_idx = m_glob0 * FANIN + np.arange(MC * FANIN)
    t = in1[e_idx].astype(np.int64)                 # [MC*16]
    wv = w1[e_idx].astype(np.float64)
    mloc = np.repeat(np.arange(MC, dtype=np.int64), FANIN)
    A1 = np.bincount(t * MP + mloc, weights=wv,
                     minlength=N_TF * MP).reshape(N_TF, MP)
    # flat layout: a1[p, tt*4096 + ch*512 + j] = A1s[ch*128+p, tt*512+j]
    a1_packed = ((A1 * A1SCALE).reshape(NCH, 128, NT, MT)
                 .transpose(1, 2, 0, 3)
                 .reshape(128, NT * NCH * MT)
                 .astype(np.float32).astype(fp8))

    b1p = np.zeros(MP, np.float32)
    b1p[:MC] = b1[m_glob0:m_glob0 + MC]
    b1c = np.ascontiguousarray(b1p.reshape(NPT, 128).T).astype(bf16)

    # --- padded per-gene weights
    w2pad = np.zeros((GP, W, W), np.float32)        # [gene, i, j]
    w2pad[:GC] = w2.reshape(N_GENES, W, W)[GC * c:GC * (c + 1)]
    b2pad = np.zeros((GP, W), np.float32)
    b2pad[:GC] = b2.reshape(N_GENES, W)[GC * c:GC * (c + 1)]
    w3pad = np.zeros((GP, W), np.float32)
    w3pad[:GC] = w3.reshape(N_GENES, W)[GC * c:GC * (c + 1)]
    b3pad = np.zeros(GP, np.float32)
    b3pad[:GC] = b3[GC * c:GC * (c + 1)]

    # --- compact W2: w2c[tt, p=(a,j), q*4+i] = w2[(tt*4+q)*32+a, i, j];
    # the kernel expands it to the block-diag stationary on device
    w2c = (w2pad.reshape(NT, 4, 32, W, W)
           .transpose(0, 2, 4, 1, 3)       # [tt, a, j, q, i]
           .reshape(NT, 128, 16))

    # --- W3 mats: W3m[pt, (a,i), b] = d(a==b) w3[g*4+i]
    idx = np.arange(32)
    W3m = np.zeros((NPT, 32, W, 32), np.float32)
    W3m[:, idx, :, idx] = w3pad.reshape(NPT, 32, W).transpose(1, 0, 2)
    w3m = W3m.reshape(NT, 4, 128, 32)   # [tt, q, (a,j), b]

    # fused per-tile weight stream: 16 compact w2 cols + 128 w3 cols
    w23 = np.zeros((NT, 128, 144), np.float32)
    w23[:, :, :16] = w2c
    w23[:, :, 16:] = w3m.transpose(0, 2, 1, 3).reshape(NT, 128, 128)
    w23m = np.ascontiguousarray(
        w23.transpose(1, 0, 2).reshape(128, NT * 144)).astype(bf16)

    # --- bias columns
    b2c = (b2pad.reshape(NPT, 32, W).transpose(1, 2, 0)
           .reshape(128, NPT).astype(np.float32))
    b3c = np.ascontiguousarray(b3pad.reshape(NT, 128).T)

    return {
        "a1": a1_packed,
        "b1c": b1c,
        "w23m": w23m,
        "b2c": b2c,
        "b3c": b3c,
    }


def _run(inputs, trace=False):
    global _COMPILED
    if _COMPILED is None:
        _COMPILED = _build_program()
    nc = _COMPILED

    bf16 = _np_bf16()
    features = np.asarray(inputs["features"], np.float32)
    w1 = np.asarray(inputs["w1"], np.float32)
    b1 = np.asarray(inputs["b1"], np.float32)
    w2 = np.asarray(inputs["w2"], np.float32)
    b2 = np.asarray(inputs["b2"], np.float32)
    w3 = np.asarray(inputs["w3"], np.float32)
    b3 = np.asarray(inputs["b3"], np.float32)
    in1 = np.asarray(inputs["in1"], np.int32)

    # stationary x: [p, ch*128 + b] = x[b, ch*128 + p]
    xt = (features.T.reshape(NCH, 128, B).transpose(1, 0, 2)
          .reshape(128, N_TF).astype(bf16))
    ident = np.eye(128, dtype=np.float32).astype(bf16)
    maskb = np.kron(np.eye(32, dtype=np.float32),
                    np.ones((4, 4), np.float32)).astype(bf16)

    in_maps = []
    for c in range(NCORES):
        m = _prep_core(c, w1, b1, w2, b2, w3, b3, in1)
        m["xt"] = xt
        m["ident"] = ident
        m["maskb"] = maskb
        in_maps.append(m)

    if trace:
        _install_ntff_shim()
    res = run_bass_kernel_spmd(nc, in_maps, core_ids=list(range(NCORES)),
                               trace=trace)
    y = np.empty((B, N_GENES), np.float32)
    for c in range(NCORES):
        yc = np.asarray(res.results[c]["out"]).astype(np.float32)  # [128,NT*128]
        # yc[p, tt*128 + b] = y[b, GC*c + tt*128 + p]
        yg = yc.reshape(128, NT, 128).transpose(2, 1, 0).reshape(B, GP)
        y[:, GC * c:GC * (c + 1)] = yg[:, :GC]
    return y, res.exec_time_ns


def kernel(**inputs) -> np.ndarray:
    y, _ = _run(inputs, trace=False)
    return y


# revision 58
# speedup vs baseline: 1.0216x; 1.0106x over previous
"""Trainium2 Bass kernel for the gene-network AE decoder (3 sparse layers).

Network (per reference):
  h1 = tanh(x @ A1 + b1)                A1: [1024, 80000], 16 nnz/col
  h2 = tanh(blockdiag4x4(W2) h1 + b2)   gene-local 4x4 mixing
  y  = blockdiag1x4(W3) h2 + b3         gene-local 4->1 reduction

Sharding: genes across the 8 cores (2500 genes -> padded to 2560 = 10240
nodes = 20 matmul tiles of 512). No inter-core communication: layer 1 only
needs the (replicated) 1024 TF features; layers 2/3 are gene-local.

The layer-1 sparse matrix is expanded to dense fp8 e3m4 on the host
(placement of the runtime w1 values at positions given by the runtime in1
indices; all arithmetic happens on device). e3m4 halves the dominant HBM
stream; w2/w3 stay bf16 (total quantization error ~1.5%, gate 2%).

Pipeline (per m-tile of 512 nodes = 128 genes), software-pipelined with
one-iteration slack between engine stages:

  t=tt:   L1   (PE)  8 chunk matmuls, xt stationary / a1 moving -> ps1
  t=tt+1: EVAC (ACT) ps1/8 -> s1 bf16;  T (PE, after L1(t)) -> psT [m,b];
          W2-expand (Pool) compact 16-col w2 -> 512-col block-diag
  t=tt+2: ADD1 (DVE) +b1;  TANH1 (ACT) -> h1T
  t=tt+3: L2   (PE)  4 block-diag W2 matmuls; ADD2 (DVE) +b2;
          TANH2 (ACT) -> h2T
  t=tt+4: L3   (PE)  4 W3 matmuls packed in one PSUM tile; ADD3 (DVE) +b3
          into a 2-tile output buffer
  t=tt+5: out DMA for each tile pair (odd tt)

PE is the binding engine (L1 37us + T 7 + L2 6 + L3 4 busy, ~96%
occupancy in span). Hard-won scheduling facts baked in here:
- each dma_start costs ~0.65us of sequencer issue time (DIRECT2D), so a1
  ships as 2-tile (1MB) DMAs from a flat [128, NT*4096] layout, w2/w3
  ride one fused 144-col stream (w2 ships compact and is expanded into
  its 97%-zeros block-diagonal form by the otherwise-idle Pool engine:
  mask x broadcast multiply), and outputs pair up 2 tiles per DMA;
- the sync HW queue starts ~2.4us before the scalar one, so xt leads the
  sync queue ahead of the a1 stream;
- a DMA whose semaphore wait isn't already satisfied blocks its queue's
  sequencer in-order, so the out DMA fires one iteration AFTER its data
  is complete, and the sync queue carries nothing but the a1 stream;
- the DMA XBAR transpose (dma_start_transpose) is NOT free - it occupies
  the issuing engine ~1.1us per [128,512] - so transposes stay on PE;
- fp8 e3m4 runs at 1 cycle/row (same as bf16) on PE; DoubleRow (0.5
  cyc/row) requires e4m3 whose 3 mantissa bits measure 2.8-4.0% rel err
  end-to-end - over the 2% gate, so no DoubleRow;
- a dummy tanh right after the const DMAs preloads the 1.3us ACT table
  during the DMA ramp;
- ptile q3 of the last tile is pure padding: its A/B chain is skipped,
  L1(last) runs 384-wide, and L3 uses a zeroed stationary stand-in.
Note: some runs execute at a 1.2x slower DVFS point (ACT_TABLE_LOAD
canary 1539ns vs nominal 1283ns); nominal-clock time is ~73-75us.
"""

import sys
import types

import numpy as np

try:
    import ml_dtypes
except ImportError:  # pragma: no cover
    ml_dtypes = None

import concourse.bass as bass
import concourse.tile as tile
from concourse import bacc, mybir
from concourse.bass_utils import run_bass_kernel_spmd

# ---------------------------------------------------------------- constants
B = 128          # batch
N_TF = 1024      # input features (= 8 chunks of 128)
N_GENES = 20000
W = 4            # nodes per gene
FANIN = 16
NCORES = 8
GC = N_GENES // NCORES      # 2500 genes / core
GP = 2560                   # padded genes / core
MP = GP * W                 # 10240 padded nodes / core
MT = 512                    # matmul moving tile (1 PSUM bank of f32)
NT = MP // MT               # 20 tiles / core
NCH = N_TF // 128           # 8 contraction chunks
A1SCALE = 8.0    # fp8e3 pre-scale: keeps w1 out of the e3m4 subnormal range
NPT = MP // 128             # 80 ptiles (128 nodes = 32 genes)

BF16 = mybir.dt.bfloat16
F32 = mybir.dt.float32
FP8 = mybir.dt.float8e3

_COMPILED = None


def _np_bf16():
    assert ml_dtypes is not None, "ml_dtypes required for bf16 host arrays"
    return ml_dtypes.bfloat16


def _np_fp8():
    assert ml_dtypes is not None, "ml_dtypes required for fp8 host arrays"
    return ml_dtypes.float8_e3m4


# ---------------------------------------------------------------- NTFF shim
def _install_ntff_shim():
    """Register the NTFF profile hook if this image's antenv lacks it."""
    try:
        import antenv
        if "antenv.axon_hooks" in sys.modules:
            return
        mod = types.ModuleType("antenv.axon_hooks")
        mod._hook = None
        mod.set_axon_ntff_profile_hook = lambda h: setattr(mod, "_hook", h)
        mod.get_axon_ntff_profile_hook = lambda: mod._hook
        sys.modules["antenv.axon_hooks"] = mod
        antenv.axon_hooks = mod
        from trn_agent_boot.trn_boot import _ntff_profile_via_ctypes
        mod.set_axon_ntff_profile_hook(
            _ntff_profile_via_ctypes("/opt/axon/libaxon_pjrt.so"))
    except Exception:
        pass


# ---------------------------------------------------------------- program
def _build_program():
    nc = bacc.Bacc("TRN2", target_bir_lowering=False, debug=False,
                   num_devices=NCORES)

    a1_ext = nc.dram_tensor("a1", [128, NT * NCH * MT], FP8,
                            kind="ExternalInput")
    xt_ext = nc.dram_tensor("xt", [128, N_TF], BF16, kind="ExternalInput")
    b1_ext = nc.dram_tensor("b1c", [128, NPT], BF16, kind="ExternalInput")
    w23_ext = nc.dram_tensor("w23m", [128, NT * 144], BF16,
                             kind="ExternalInput")
    mask_ext = nc.dram_tensor("maskb", [128, 128], BF16,
                              kind="ExternalInput")
    b2_ext = nc.dram_tensor("b2c", [128, NPT], F32, kind="ExternalInput")
    b3_ext = nc.dram_tensor("b3c", [128, NT], F32, kind="ExternalInput")
    id_ext = nc.dram_tensor("ident", [128, 128], BF16, kind="ExternalInput")
    # out[p, tt*128 + b] = y[b, gene tt*128 + p] (tile-major columns)
    out_ext = nc.dram_tensor("out", [128, NT * 128], BF16,
                             kind="ExternalOutput")

    LAST = NT - 1

    with tile.TileContext(nc) as tc:
        with (
            tc.tile_pool(name="consts", bufs=1) as consts,
            tc.tile_pool(name="a1p", bufs=4) as a1p,
            tc.tile_pool(name="w2sp", bufs=7) as w2sp,
            tc.tile_pool(name="w2xp", bufs=4) as w2xp,
            tc.tile_pool(name="ps1p", bufs=2, space="PSUM") as ps1p,
            tc.tile_pool(name="ps2p", bufs=2, space="PSUM") as ps2p,
            tc.tile_pool(name="ps3p", bufs=2, space="PSUM") as ps3p,
            tc.tile_pool(name="s1p", bufs=3) as s1p,
            tc.tile_pool(name="s1Tp", bufs=2, space="PSUM") as s1Tp,
            tc.tile_pool(name="s2p", bufs=3) as s2p,
            tc.tile_pool(name="s3p", bufs=3) as s3p,
            tc.tile_pool(name="h1Tp", bufs=3) as h1Tp,
            tc.tile_pool(name="h2Tp", bufs=3) as h2Tp,
            tc.tile_pool(name="outp", bufs=3) as outp,
        ):
            # the sync HW queue starts executing ~2.4us before the scalar
            # one, so xt leads the sync queue ahead of the a1 stream (both
            # gate the first matmul); consts ride the late scalar queue,
            # ident first (T(0) needs it soonest)
            xt = consts.tile([128, N_TF], BF16, tag="xt")
            nc.sync.dma_start(xt[:], xt_ext.ap())
            ident = consts.tile([128, 128], BF16, tag="ident")
            nc.scalar.dma_start(ident[:], id_ext.ap())
            b1c = consts.tile([128, NPT], BF16, tag="b1c")
            nc.scalar.dma_start(b1c[:], b1_ext.ap())
            b2c = consts.tile([128, NPT], F32, tag="b2c")
            nc.scalar.dma_start(b2c[:], b2_ext.ap())
            b3c = consts.tile([128, NT], F32, tag="b3c")
            nc.scalar.dma_start(b3c[:], b3_ext.ap())
            # 4x4 block-diagonal ones mask for the on-device w2 expansion
            maskb = consts.tile([128, 128], BF16, tag="maskb")
            nc.scalar.dma_start(maskb[:], mask_ext.ap())
            # stand-in for the all-padding ptile q3 of the last tile
            zero_h2 = consts.tile([128, 128], BF16, tag="zero_h2")
            nc.gpsimd.memset(zero_h2[:], 0)
            # preload the tanh ACT table during the DMA ramp so the first
            # real tanh doesn't eat the 1.3us table switch (reads xt: it
            # lands early on the sync queue)
            warm = consts.tile([128, 1], BF16, tag="warm")
            nc.scalar.activation(warm[:], xt[:, :1],
                                 mybir.ActivationFunctionType.Tanh)

            st = {}   # tile index -> dict of live tensors

            def stageW(tt):
                """expand compact w2 (16 cols) into the 512-col block-diag
                stationary on the idle Pool engine: 1 iter after its DMA."""
                d = st.setdefault(tt, {})
                w23t = d["w23t"]
                w2x = w2xp.tile([128, 512], BF16, tag="w2x",
                                name=f"w2x_{tt}")
                nc.gpsimd.tensor_tensor(
                    w2x[:].rearrange("p (q b i) -> p q b i", q=4, b=32),
                    maskb[:].rearrange("p (b i) -> p b i", b=32)[
                        :, None, :, :].to_broadcast([128, 4, 32, 4]),
                    w23t[:, :16].rearrange("p (q i) -> p q i", q=4)[
                        :, :, None, :].to_broadcast([128, 4, 32, 4]),
                    mybir.AluOpType.mult)
                d["w2x"] = w2x

            def stageA1e(tt, q0=0, q1=4):
                """evac (ACT): 1 iter after L1."""
                d = st[tt]
                nq = q1 - q0
                sfx = f"{tt}_{q0}"
                s1 = s1p.tile([128, nq * 128], BF16, tag="s1",
                              name=f"s1_{sfx}")
                nc.scalar.activation(s1[:], d["ps1"][:, q0 * 128:q1 * 128],
                                     mybir.ActivationFunctionType.Copy,
                                     scale=1.0 / A1SCALE)
                d[f"s1_{q0}"] = s1

            def stageA1t(tt, q0=0, q1=4):
                """transpose (PE): 1 iter after L1, placed after L1(t) in
                the PE stream so the evac has most of an iteration of
                slack."""
                d = st[tt]
                nq = q1 - q0
                sfx = f"{tt}_{q0}"
                s1 = d.pop(f"s1_{q0}")
                psT = s1Tp.tile([128, nq, 128], BF16, tag="psT",
                                name=f"psT_{sfx}")
                for q in range(nq):
                    nc.tensor.transpose(psT[:, q, :],
                                        s1[:, q * 128:(q + 1) * 128],
                                        ident[:])
                d[f"s1T_{q0}"] = psT

            def stageA2(tt, q0=0, q1=4):
                """+b1 (DVE, broadcast over batch) + tanh (ACT): 2 iters
                after L1."""
                d = st[tt]
                nq = q1 - q0
                sfx = f"{tt}_{q0}"
                s1T = d.pop(f"s1T_{q0}")
                s2 = s2p.tile([128, nq * 128], BF16, tag="s2",
                              name=f"s2_{sfx}")
                nc.vector.tensor_tensor(
                    s2[:].rearrange("p (q b) -> p q b", q=nq),
                    s1T[:],
                    b1c[:, tt * 4 + q0:tt * 4 + q1, None].to_broadcast(
                        [128, nq, 128]),
                    mybir.AluOpType.add)
                h1T = h1Tp.tile([128, nq * 128], BF16, tag="h1T",
                                name=f"h1T_{sfx}")
                nc.scalar.activation(h1T[:], s2[:],
                                     mybir.ActivationFunctionType.Tanh)
                d[f"h1T_{q0}"] = h1T

            def stageB(tt, q0=0, q1=4):
                """layer 2 (PE) + bias (DVE) + tanh (ACT): 3 iters after
                L1."""
                d = st[tt]
                nq = q1 - q0
                sfx = f"{tt}_{q0}"
                h1T = d.pop(f"h1T_{q0}")
                ps2 = ps2p.tile([128, nq * 128], F32, tag="ps2",
                                name=f"ps2_{sfx}")
                for q in range(nq):
                    nc.tensor.matmul(
                        ps2[:, q * 128:(q + 1) * 128],
                        d["w2x"][:, (q0 + q) * 128:(q0 + q + 1) * 128],
                        h1T[:, q * 128:(q + 1) * 128],
                        start=True, stop=True)
                s3 = s3p.tile([128, nq * 128], F32, tag="s3",
                              name=f"s3_{sfx}")
                nc.vector.tensor_tensor(
                    s3[:].rearrange("p (q b) -> p q b", q=nq),
                    ps2[:].rearrange("p (q b) -> p q b", q=nq),
                    b2c[:, tt * 4 + q0:tt * 4 + q1, None].to_broadcast(
                        [128, nq, 128]),
                    mybir.AluOpType.add)
                h2T = h2Tp.tile([128, nq * 128], BF16, tag="h2T",
                                name=f"h2T_{sfx}")
                nc.scalar.activation(h2T[:], s3[:],
                                     mybir.ActivationFunctionType.Tanh)
                d[f"h2T_{q0}"] = h2T

            def stageC(tt):
                """layer 3 (PE, full width) + b3 (DVE): 4 iters after L1."""
                d = st[tt]
                ps3 = ps3p.tile([128, 128], F32, tag="ps3",
                                name=f"ps3_{tt}")
                for q in range(4):
                    if tt == LAST:
                        # ptile q3 of the last tile is all padding
                        h2q = (zero_h2[:] if q == 3
                               else d.pop(f"h2T_{q}")[:, :128])
                    elif q == 0:
                        d["_h2T"] = d.pop("h2T_0")
                        h2q = d["_h2T"][:, :128]
                    else:
                        h2q = d["_h2T"][:, q * 128:(q + 1) * 128]
                    nc.tensor.matmul(
                        ps3[q * 32:(q + 1) * 32, :],
                        d["w3t"][:, q * 32:(q + 1) * 32],
                        h2q,
                        start=True, stop=True,
                        tile_position=(0, 32 * q))
                d.pop("_h2T", None)
                if tt % 2 == 0:
                    yt = outp.tile([128, 256], BF16, tag="yt",
                                   name=f"yt_{tt // 2}")
                    st[tt + 1]["ytbuf"] = yt
                else:
                    yt = d.pop("ytbuf")
                nc.vector.tensor_scalar_add(
                    yt[:, (tt % 2) * 128:(tt % 2 + 1) * 128], ps3[:],
                    b3c[:128, tt:tt + 1])
                d["yt"] = yt

            def stageD(tt):
                """out DMA for tile pair (tt-1, tt), odd tt: 5 iters after
                L1, one iteration after its yt half is written so the sem
                wait never blocks the queue sequencer."""
                if tt % 2 == 0:
                    return
                yt = st[tt].pop("yt")
                nc.scalar.dma_start(
                    out_ext.ap()[:, (tt - 1) * 128:(tt + 1) * 128], yt[:])

            def run(stage, tl, quarters=True):
                if 0 <= tl < NT:
                    if tl == LAST and quarters:
                        # q3 is all padding: skip its whole A/B chain
                        for q in range(3):
                            stage(tl, q, q + 1)
                    else:
                        stage(tl)

            for t in range(NT + 5):
                # ---- input DMAs (each dma_start costs ~0.65us of
                # sequencer issue time, so a1 rides in 2-tile chunks and
                # w2/w3 are fused into one 640-col stream)
                TW = NCH * MT   # 4096 a1 cols per m-tile
                if t < NT and t % 2 == 0:
                    a1t = a1p.tile([128, 2 * TW], FP8,
                                   tag="a1t", name=f"a1t_{t // 2}")
                    if t == 0:
                        # chunks 0-1 first so L1(0) starts after 128KB
                        nc.sync.dma_start(a1t[:, :2 * MT],
                                          a1_ext.ap()[:, :2 * MT])
                        nc.sync.dma_start(a1t[:, 2 * MT:TW],
                                          a1_ext.ap()[:, 2 * MT:TW])
                        nc.sync.dma_start(a1t[:, TW:],
                                          a1_ext.ap()[:, TW:2 * TW])
                    else:
                        nc.sync.dma_start(
                            a1t[:], a1_ext.ap()[:, t * TW:(t + 2) * TW])
                    st.setdefault(t, {})["a1t"] = a1t
                    st.setdefault(t + 1, {})["a1t"] = a1t
                if t < NT:
                    # compact stream: 16 w2 cols (4x4 per gene-node) + 128
                    # dense w3 cols; w2 is expanded on the idle Pool engine
                    w23t = w2sp.tile([128, 144], BF16, tag="w23t",
                                     name=f"w23t_{t}")
                    nc.scalar.dma_start(
                        w23t[:], w23_ext.ap()[:, t * 144:(t + 1) * 144])

                # engine-stream order per iteration:
                #   ACT: evac(t-1), tanh1(t-2), tanh2(t-3)
                #   DVE: add1(t-2), add2(t-3), add3(t-4)
                #   PE:  L2(t-3), L1(t), T(t-1), L3(t-4) — L2 before L1 so
                #        PE does ready work while a1(t) may still be in
                #        flight (L1-first measured 2us slower)
                run(stageA1e, t - 1)
                run(stageW, t - 1, quarters=False)
                run(stageA2, t - 2)
                run(stageB, t - 3)

                if t < NT:
                    ps1 = ps1p.tile([128, MT], F32, tag="ps1",
                                    name=f"ps1_{t}")
                    a1v = st[t]
                    a1v["ps1"] = ps1
                    a1v["w23t"] = w23t[:]
                    a1v["w3t"] = w23t[:, 16:144]
                    a1m = a1v.pop("a1t")
                    off = (t % 2) * TW
                    mw = 384 if t == LAST else MT  # last ptile is padding
                    for ch in range(NCH):
                        nc.tensor.matmul(
                            ps1[:, :mw],
                            xt[:, ch * 128:(ch + 1) * 128],
                            a1m[:, off + ch * MT:off + ch * MT + mw],
                            start=(ch == 0), stop=(ch == NCH - 1))

                run(stageA1t, t - 1)
                run(stageC, t - 4, quarters=False)
                run(stageD, t - 5, quarters=False)

    nc.compile()
    return nc


# ---------------------------------------------------------------- host prep
def _prep_core(c, w1, b1, w2, b2, w3, b3, in1):
    """Build the per-core input arrays (index/layout placement only)."""
    bf16 = _np_bf16()
    fp8 = _np_fp8()
    MC = GC * W  # 10000 real nodes per core

    # --- layer-1 dense matrix [1024, MP], columns = local node id 4g+j
    m_glob0 = (GC * c) * W
    e_idx = m_glob0 * FANIN + np.arange(MC * FANIN)
    t = in1[e_idx].astype(np.int64)                 # [MC*16]
    wv = w1[e_idx].astype(np.float64)
    mloc = np.repeat(np.arange(MC, dtype=np.int64), FANIN)
    A1 = np.bincount(t * MP + mloc, weights=wv,
                     minlength=N_TF * MP).reshape(N_TF, MP)
    # flat layout: a1[p, tt*4096 + ch*512 + j] = A1s[ch*128+p, tt*512+j]
    a1_packed = ((A1 * A1SCALE).reshape(NCH, 128, NT, MT)
                 .transpose(1, 2, 0, 3)
                 .reshape(128, NT * NCH * MT)
                 .astype(np.float32).astype(fp8))

    b1p = np.zeros(MP, np.float32)
    b1p[:MC] = b1[m_glob0:m_glob0 + MC]
    b1c = np.ascontiguousarray(b1p.reshape(NPT, 128).T).astype(bf16)

    # --- padded per-gene weights
    w2pad = np.zeros((GP, W, W), np.float32)        # [gene, i, j]
    w2pad[:GC] = w2.reshape(N_GENES, W, W)[GC * c:GC * (c + 1)]
    b2pad = np.zeros((GP, W), np.float32)
    b2pad[:GC] = b2.reshape(N_GENES, W)[GC * c:GC * (c + 1)]
    w3pad = np.zeros((GP, W), np.float32)
    w3pad[:GC] = w3.reshape(N_GENES, W)[GC * c:GC * (c + 1)]
    b3pad = np.zeros(GP, np.float32)
    b3pad[:GC] = b3[GC * c:GC * (c + 1)]

    # --- compact W2: w2c[tt, p=(a,j), q*4+i] = w2[(tt*4+q)*32+a, i, j];
    # the kernel expands it to the block-diag stationary on device
    w2c = (w2pad.reshape(NT, 4, 32, W, W)
           .transpose(0, 2, 4, 1, 3)       # [tt, a, j, q, i]
           .reshape(NT, 128, 16))

    # --- W3 mats: W3m[pt, (a,i), b] = d(a==b) w3[g*4+i]
    idx = np.arange(32)
    W3m = np.zeros((NPT, 32, W, 32), np.float32)
    W3m[:, idx, :, idx] = w3pad.reshape(NPT, 32, W).transpose(1, 0, 2)
    w3m = W3m.reshape(NT, 4, 128, 32)   # [tt, q, (a,j), b]

    # fused per-tile weight stream: 16 compact w2 cols + 128 w3 cols
    w23 = np.zeros((NT, 128, 144), np.float32)
    w23[:, :, :16] = w2c
    w23[:, :, 16:] = w3m.transpose(0, 2, 1, 3).reshape(NT, 128, 128)
    w23m = np.ascontiguousarray(
        w23.transpose(1, 0, 2).reshape(128, NT * 144)).astype(bf16)

    # --- bias columns
    b2c = (b2pad.reshape(NPT, 32, W).transpose(1, 2, 0)
           .reshape(128, NPT).astype(np.float32))
    b3c = np.ascontiguousarray(b3pad.reshape(NT, 128).T)

    return {
        "a1": a1_packed,
        "b1c": b1c,
        "w23m": w23m,
        "b2c": b2c,
        "b3c": b3c,
    }


def _run(inputs, trace=False):
    global _COMPILED
    if _COMPILED is None:
        _COMPILED = _build_program()
    nc = _COMPILED

    bf16 = _np_bf16()
    features = np.asarray(inputs["features"], np.float32)
    w1 = np.asarray(inputs["w1"], np.float32)
    b1 = np.asarray(inputs["b1"], np.float32)
    w2 = np.asarray(inputs["w2"], np.float32)
    b2 = np.asarray(inputs["b2"], np.float32)
    w3 = np.asarray(inputs["w3"], np.float32)
    b3 = np.asarray(inputs["b3"], np.float32)
    in1 = np.asarray(inputs["in1"], np.int32)

    # stationary x: [p, ch*128 + b] = x[b, ch*128 + p]
    xt = (features.T.reshape(NCH, 128, B).transpose(1, 0, 2)
          .reshape(128, N_TF).astype(bf16))
    ident = np.eye(128, dtype=np.float32).astype(bf16)
    maskb = np.kron(np.eye(32, dtype=np.float32),
                    np.ones((4, 4), np.float32)).astype(bf16)

    in_maps = []
    for c in range(NCORES):
        m = _prep_core(c, w1, b1, w2, b2, w3, b3, in1)
        m["xt"] = xt
        m["ident"] = ident
        m["maskb"] = maskb
        in_maps.append(m)

    if trace:
        _install_ntff_shim()
    res = run_bass_kernel_spmd(nc, in_maps, core_ids=list(range(NCORES)),
                               trace=trace)
    y = np.empty((B, N_GENES), np.float32)
    for c in range(NCORES):
        yc = np.asarray(res.results[c]["out"]).astype(np.float32)  # [128,NT*128]
        # yc[p, tt*128 + b] = y[b, GC*c + tt*128 + p]
        yg = yc.reshape(128, NT, 128).transpose(2, 1, 0).reshape(B, GP)
        y[:, GC * c:GC * (c + 1)] = yg[:, :GC]
    return y, res.exec_time_ns


def kernel(**inputs) -> np.ndarray:
    y, _ = _run(inputs, trace=False)
    return y


# revision 60
# speedup vs baseline: 1.0302x; 1.0084x over previous
"""Trainium2 Bass kernel for the gene-network AE decoder (3 sparse layers).

Network (per reference):
  h1 = tanh(x @ A1 + b1)                A1: [1024, 80000], 16 nnz/col
  h2 = tanh(blockdiag4x4(W2) h1 + b2)   gene-local 4x4 mixing
  y  = blockdiag1x4(W3) h2 + b3         gene-local 4->1 reduction

Sharding: genes across the 8 cores (2500 genes -> padded to 2560 = 10240
nodes = 20 matmul tiles of 512). No inter-core communication: layer 1 only
needs the (replicated) 1024 TF features; layers 2/3 are gene-local.

The layer-1 sparse matrix is expanded to dense fp8 e3m4 on the host
(placement of the runtime w1 values at positions given by the runtime in1
indices; all arithmetic happens on device). e3m4 halves the dominant HBM
stream; w2/w3 stay bf16 (total quantization error ~1.5%, gate 2%).

Pipeline (per m-tile of 512 nodes = 128 genes), software-pipelined with
one-iteration slack between engine stages:

  t=tt:   L1   (PE)  8 chunk matmuls, xt stationary / a1 moving -> ps1
  t=tt+1: EVAC (ACT) ps1/8 -> s1 bf16;  T (PE, after L1(t)) -> psT [m,b];
          W2-expand (Pool) compact 16-col w2 -> 512-col block-diag
  t=tt+2: ADD1 (DVE) +b1;  TANH1 (ACT) -> h1T
  t=tt+3: L2   (PE)  4 block-diag W2 matmuls; ADD2 (DVE) +b2;
          TANH2 (ACT) -> h2T
  t=tt+4: L3   (PE)  4 W3 matmuls packed in one PSUM tile; ADD3 (DVE) +b3
          into a 2-tile output buffer
  t=tt+5: out DMA for each tile pair (odd tt)

PE is the binding engine (L1 37us + T 7 + L2 6 + L3 4 busy, ~96%
occupancy in span). Hard-won scheduling facts baked in here:
- each dma_start costs ~0.65us of sequencer issue time (DIRECT2D), so a1
  ships as 2-tile (1MB) DMAs from a flat [128, NT*4096] layout, w2/w3
  ride one fused 144-col stream (w2 ships compact and is expanded into
  its 97%-zeros block-diagonal form by the otherwise-idle Pool engine:
  mask x broadcast multiply), and outputs pair up 2 tiles per DMA;
- the sync HW queue starts ~2.4us before the scalar one, so xt leads the
  sync queue ahead of the a1 stream;
- a DMA whose semaphore wait isn't already satisfied blocks its queue's
  sequencer in-order, so the out DMA fires one iteration AFTER its data
  is complete, and the sync queue carries nothing but the a1 stream;
- the DMA XBAR transpose (dma_start_transpose) is NOT free - it occupies
  the issuing engine ~1.1us per [128,512] - so transposes stay on PE;
- fp8 e3m4 runs at 1 cycle/row (same as bf16) on PE; DoubleRow (0.5
  cyc/row) requires e4m3 whose 3 mantissa bits measure 2.8-4.0% rel err
  end-to-end - over the 2% gate, so no DoubleRow;
- a dummy tanh right after the const DMAs preloads the 1.3us ACT table
  during the DMA ramp;
- ptile q3 of the last tile is pure padding: its A/B chain is skipped,
  L1(last) runs 384-wide, and L3 uses a zeroed stationary stand-in.
Note: some runs execute at a 1.2x slower DVFS point (ACT_TABLE_LOAD
canary 1539ns vs nominal 1283ns); nominal-clock time is ~73-75us.
"""

import sys
import types

import numpy as np

try:
    import ml_dtypes
except ImportError:  # pragma: no cover
    ml_dtypes = None

import concourse.bass as bass
import concourse.tile as tile
from concourse import bacc, mybir
from concourse.bass_utils import run_bass_kernel_spmd

# ---------------------------------------------------------------- constants
B = 128          # batch
N_TF = 1024      # input features (= 8 chunks of 128)
N_GENES = 20000
W = 4            # nodes per gene
FANIN = 16
NCORES = 8
GC = N_GENES // NCORES      # 2500 genes / core
GP = 2560                   # padded genes / core
MP = GP * W                 # 10240 padded nodes / core
MT = 512                    # matmul moving tile (1 PSUM bank of f32)
NT = MP // MT               # 20 tiles / core
NCH = N_TF // 128           # 8 contraction chunks
A1SCALE = 8.0    # fp8e3 pre-scale: keeps w1 out of the e3m4 subnormal range
NPT = MP // 128             # 80 ptiles (128 nodes = 32 genes)

BF16 = mybir.dt.bfloat16
F32 = mybir.dt.float32
FP8 = mybir.dt.float8e3

_COMPILED = None


def _np_bf16():
    assert ml_dtypes is not None, "ml_dtypes required for bf16 host arrays"
    return ml_dtypes.bfloat16


def _np_fp8():
    assert ml_dtypes is not None, "ml_dtypes required for fp8 host arrays"
    return ml_dtypes.float8_e3m4


# ---------------------------------------------------------------- NTFF shim
def _install_ntff_shim():
    """Register the NTFF profile hook if this image's antenv lacks it."""
    try:
        import antenv
        if "antenv.axon_hooks" in sys.modules:
            return
        mod = types.ModuleType("antenv.axon_hooks")
        mod._hook = None
        mod.set_axon_ntff_profile_hook = lambda h: setattr(mod, "_hook", h)
        mod.get_axon_ntff_profile_hook = lambda: mod._hook
        sys.modules["antenv.axon_hooks"] = mod
        antenv.axon_hooks = mod
        from trn_agent_boot.trn_boot import _ntff_profile_via_ctypes
        mod.set_axon_ntff_profile_hook(
            _ntff_profile_via_ctypes("/opt/axon/libaxon_pjrt.so"))
    except Exception:
        pass


# ---------------------------------------------------------------- program
def _build_program():
    nc = bacc.Bacc("TRN2", target_bir_lowering=False, debug=False,
                   num_devices=NCORES)

    a1_ext = nc.dram_tensor("a1", [128, NT * NCH * MT], FP8,
                            kind="ExternalInput")
    xt_ext = nc.dram_tensor("xt", [128, N_TF], BF16, kind="ExternalInput")
    b1_ext = nc.dram_tensor("b1c", [128, NPT], BF16, kind="ExternalInput")
    w23_ext = nc.dram_tensor("w23m", [128, NT * 144], BF16,
                             kind="ExternalInput")
    mask_ext = nc.dram_tensor("maskb", [128, 128], BF16,
                              kind="ExternalInput")
    b2_ext = nc.dram_tensor("b2c", [128, NPT], F32, kind="ExternalInput")
    b3_ext = nc.dram_tensor("b3c", [128, NT], F32, kind="ExternalInput")
    id_ext = nc.dram_tensor("ident", [128, 128], BF16, kind="ExternalInput")
    # out[p, tt*128 + b] = y[b, gene tt*128 + p] (tile-major columns)
    out_ext = nc.dram_tensor("out", [128, NT * 128], BF16,
                             kind="ExternalOutput")

    LAST = NT - 1

    with tile.TileContext(nc) as tc:
        with (
            tc.tile_pool(name="consts", bufs=1) as consts,
            tc.tile_pool(name="a1p", bufs=4) as a1p,
            tc.tile_pool(name="w2sp", bufs=7) as w2sp,
            tc.tile_pool(name="w2xp", bufs=4) as w2xp,
            tc.tile_pool(name="ps1p", bufs=2, space="PSUM") as ps1p,
            tc.tile_pool(name="ps2p", bufs=2, space="PSUM") as ps2p,
            tc.tile_pool(name="ps3p", bufs=2, space="PSUM") as ps3p,
            tc.tile_pool(name="s1p", bufs=3) as s1p,
            tc.tile_pool(name="s1Tp", bufs=2, space="PSUM") as s1Tp,
            tc.tile_pool(name="s2p", bufs=3) as s2p,
            tc.tile_pool(name="s3p", bufs=3) as s3p,
            tc.tile_pool(name="h1Tp", bufs=3) as h1Tp,
            tc.tile_pool(name="h2Tp", bufs=3) as h2Tp,
            tc.tile_pool(name="outp", bufs=3) as outp,
        ):
            # the sync HW queue starts executing ~2.4us before the scalar
            # one, so xt leads the sync queue ahead of the a1 stream (both
            # gate the first matmul); consts ride the late scalar queue,
            # ident first (T(0) needs it soonest)
            # first 2 xt chunks ahead of a1 chunks 0-1 so L1(0) starts
            # after only 192KB; the rest streams behind the first matmuls
            xt = consts.tile([128, N_TF], BF16, tag="xt")
            nc.sync.dma_start(xt[:, :256], xt_ext.ap()[:, :256])
            ident = consts.tile([128, 128], BF16, tag="ident")
            nc.scalar.dma_start(ident[:], id_ext.ap())
            b1c = consts.tile([128, NPT], BF16, tag="b1c")
            nc.scalar.dma_start(b1c[:], b1_ext.ap())
            b2c = consts.tile([128, NPT], F32, tag="b2c")
            nc.scalar.dma_start(b2c[:], b2_ext.ap())
            b3c = consts.tile([128, NT], F32, tag="b3c")
            nc.scalar.dma_start(b3c[:], b3_ext.ap())
            # 4x4 block-diagonal ones mask for the on-device w2 expansion
            maskb = consts.tile([128, 128], BF16, tag="maskb")
            nc.scalar.dma_start(maskb[:], mask_ext.ap())
            # stand-in for the all-padding ptile q3 of the last tile
            zero_h2 = consts.tile([128, 128], BF16, tag="zero_h2")
            nc.gpsimd.memset(zero_h2[:], 0)
            # preload the tanh ACT table during the DMA ramp so the first
            # real tanh doesn't eat the 1.3us table switch (reads xt: it
            # lands early on the sync queue)
            warm = consts.tile([128, 1], BF16, tag="warm")
            nc.scalar.activation(warm[:], xt[:, :1],
                                 mybir.ActivationFunctionType.Tanh)

            st = {}   # tile index -> dict of live tensors

            def stageW(tt):
                """expand compact w2 (16 cols) into the 512-col block-diag
                stationary on the idle Pool engine: 1 iter after its DMA."""
                d = st.setdefault(tt, {})
                w23t = d["w23t"]
                w2x = w2xp.tile([128, 512], BF16, tag="w2x",
                                name=f"w2x_{tt}")
                nc.gpsimd.tensor_tensor(
                    w2x[:].rearrange("p (q b i) -> p q b i", q=4, b=32),
                    maskb[:].rearrange("p (b i) -> p b i", b=32)[
                        :, None, :, :].to_broadcast([128, 4, 32, 4]),
                    w23t[:, :16].rearrange("p (q i) -> p q i", q=4)[
                        :, :, None, :].to_broadcast([128, 4, 32, 4]),
                    mybir.AluOpType.mult)
                d["w2x"] = w2x

            def stageA1e(tt, q0=0, q1=4):
                """evac (ACT): 1 iter after L1."""
                d = st[tt]
                nq = q1 - q0
                sfx = f"{tt}_{q0}"
                s1 = s1p.tile([128, nq * 128], BF16, tag="s1",
                              name=f"s1_{sfx}")
                nc.scalar.activation(s1[:], d["ps1"][:, q0 * 128:q1 * 128],
                                     mybir.ActivationFunctionType.Copy,
                                     scale=1.0 / A1SCALE)
                d[f"s1_{q0}"] = s1

            def stageA1t(tt, q0=0, q1=4):
                """transpose (PE): 1 iter after L1, placed after L1(t) in
                the PE stream so the evac has most of an iteration of
                slack."""
                d = st[tt]
                nq = q1 - q0
                sfx = f"{tt}_{q0}"
                s1 = d.pop(f"s1_{q0}")
                psT = s1Tp.tile([128, nq, 128], BF16, tag="psT",
                                name=f"psT_{sfx}")
                for q in range(nq):
                    nc.tensor.transpose(psT[:, q, :],
                                        s1[:, q * 128:(q + 1) * 128],
                                        ident[:])
                d[f"s1T_{q0}"] = psT

            def stageA2(tt, q0=0, q1=4):
                """+b1 (DVE, broadcast over batch) + tanh (ACT): 2 iters
                after L1."""
                d = st[tt]
                nq = q1 - q0
                sfx = f"{tt}_{q0}"
                s1T = d.pop(f"s1T_{q0}")
                s2 = s2p.tile([128, nq * 128], BF16, tag="s2",
                              name=f"s2_{sfx}")
                nc.vector.tensor_tensor(
                    s2[:].rearrange("p (q b) -> p q b", q=nq),
                    s1T[:],
                    b1c[:, tt * 4 + q0:tt * 4 + q1, None].to_broadcast(
                        [128, nq, 128]),
                    mybir.AluOpType.add)
                h1T = h1Tp.tile([128, nq * 128], BF16, tag="h1T",
                                name=f"h1T_{sfx}")
                nc.scalar.activation(h1T[:], s2[:],
                                     mybir.ActivationFunctionType.Tanh)
                d[f"h1T_{q0}"] = h1T

            def stageB(tt, q0=0, q1=4):
                """layer 2 (PE) + bias (DVE) + tanh (ACT): 3 iters after
                L1."""
                d = st[tt]
                nq = q1 - q0
                sfx = f"{tt}_{q0}"
                h1T = d.pop(f"h1T_{q0}")
                ps2 = ps2p.tile([128, nq * 128], F32, tag="ps2",
                                name=f"ps2_{sfx}")
                for q in range(nq):
                    nc.tensor.matmul(
                        ps2[:, q * 128:(q + 1) * 128],
                        d["w2x"][:, (q0 + q) * 128:(q0 + q + 1) * 128],
                        h1T[:, q * 128:(q + 1) * 128],
                        start=True, stop=True)
                s3 = s3p.tile([128, nq * 128], F32, tag="s3",
                              name=f"s3_{sfx}")
                nc.vector.tensor_tensor(
                    s3[:].rearrange("p (q b) -> p q b", q=nq),
                    ps2[:].rearrange("p (q b) -> p q b", q=nq),
                    b2c[:, tt * 4 + q0:tt * 4 + q1, None].to_broadcast(
                        [128, nq, 128]),
                    mybir.AluOpType.add)
                h2T = h2Tp.tile([128, nq * 128], BF16, tag="h2T",
                                name=f"h2T_{sfx}")
                nc.scalar.activation(h2T[:], s3[:],
                                     mybir.ActivationFunctionType.Tanh)
                d[f"h2T_{q0}"] = h2T

            def stageC(tt):
                """layer 3 (PE, full width) + b3 (DVE): 4 iters after L1."""
                d = st[tt]
                ps3 = ps3p.tile([128, 128], F32, tag="ps3",
                                name=f"ps3_{tt}")
                for q in range(4):
                    if tt == LAST:
                        # ptile q3 of the last tile is all padding
                        h2q = (zero_h2[:] if q == 3
                               else d.pop(f"h2T_{q}")[:, :128])
                    elif q == 0:
                        d["_h2T"] = d.pop("h2T_0")
                        h2q = d["_h2T"][:, :128]
                    else:
                        h2q = d["_h2T"][:, q * 128:(q + 1) * 128]
                    nc.tensor.matmul(
                        ps3[q * 32:(q + 1) * 32, :],
                        d["w3t"][:, q * 32:(q + 1) * 32],
                        h2q,
                        start=True, stop=True,
                        tile_position=(0, 32 * q))
                d.pop("_h2T", None)
                if tt % 2 == 0:
                    yt = outp.tile([128, 256], BF16, tag="yt",
                                   name=f"yt_{tt // 2}")
                    st[tt + 1]["ytbuf"] = yt
                else:
                    yt = d.pop("ytbuf")
                nc.vector.tensor_scalar_add(
                    yt[:, (tt % 2) * 128:(tt % 2 + 1) * 128], ps3[:],
                    b3c[:128, tt:tt + 1])
                d["yt"] = yt

            def stageD(tt):
                """out DMA for tile pair (tt-1, tt), odd tt: 5 iters after
                L1, one iteration after its yt half is written so the sem
                wait never blocks the queue sequencer."""
                if tt % 2 == 0:
                    return
                yt = st[tt].pop("yt")
                nc.scalar.dma_start(
                    out_ext.ap()[:, (tt - 1) * 128:(tt + 1) * 128], yt[:])

            def run(stage, tl, quarters=True):
                if 0 <= tl < NT:
                    if tl == LAST and quarters:
                        # q3 is all padding: skip its whole A/B chain
                        for q in range(3):
                            stage(tl, q, q + 1)
                    else:
                        stage(tl)

            for t in range(NT + 5):
                # ---- input DMAs (each dma_start costs ~0.65us of
                # sequencer issue time, so a1 rides in 2-tile chunks and
                # w2/w3 are fused into one 640-col stream)
                TW = NCH * MT   # 4096 a1 cols per m-tile
                if t < NT and t % 2 == 0:
                    a1t = a1p.tile([128, 2 * TW], FP8,
                                   tag="a1t", name=f"a1t_{t // 2}")
                    if t == 0:
                        # chunks 0-1 first so L1(0) starts after 128KB,
                        # then the rest of xt, then a1 chunks 2-7
                        nc.sync.dma_start(a1t[:, :2 * MT],
                                          a1_ext.ap()[:, :2 * MT])
                        nc.sync.dma_start(xt[:, 256:], xt_ext.ap()[:, 256:])
                        nc.sync.dma_start(a1t[:, 2 * MT:TW],
                                          a1_ext.ap()[:, 2 * MT:TW])
                        nc.sync.dma_start(a1t[:, TW:],
                                          a1_ext.ap()[:, TW:2 * TW])
                    else:
                        nc.sync.dma_start(
                            a1t[:], a1_ext.ap()[:, t * TW:(t + 2) * TW])
                    st.setdefault(t, {})["a1t"] = a1t
                    st.setdefault(t + 1, {})["a1t"] = a1t
                if t < NT:
                    # compact stream: 16 w2 cols (4x4 per gene-node) + 128
                    # dense w3 cols; w2 is expanded on the idle Pool engine
                    w23t = w2sp.tile([128, 144], BF16, tag="w23t",
                                     name=f"w23t_{t}")
                    nc.scalar.dma_start(
                        w23t[:], w23_ext.ap()[:, t * 144:(t + 1) * 144])

                # engine-stream order per iteration:
                #   ACT: evac(t-1), tanh1(t-2), tanh2(t-3)
                #   DVE: add1(t-2), add2(t-3), add3(t-4)
                #   PE:  L2(t-3), L1(t), T(t-1), L3(t-4) — L2 before L1 so
                #        PE does ready work while a1(t) may still be in
                #        flight (L1-first measured 2us slower)
                run(stageA1e, t - 1)
                run(stageW, t - 1, quarters=False)
                run(stageA2, t - 2)
                run(stageB, t - 3)

                if t < NT:
                    ps1 = ps1p.tile([128, MT], F32, tag="ps1",
                                    name=f"ps1_{t}")
                    a1v = st[t]
                    a1v["ps1"] = ps1
                    a1v["w23t"] = w23t[:]
                    a1v["w3t"] = w23t[:, 16:144]
                    a1m = a1v.pop("a1t")
                    off = (t % 2) * TW
                    mw = 384 if t == LAST else MT  # last ptile is padding
                    for ch in range(NCH):
                        nc.tensor.matmul(
                            ps1[:, :mw],
                            xt[:, ch * 128:(ch + 1) * 128],
                            a1m[:, off + ch * MT:off + ch * MT + mw],
                            start=(ch == 0), stop=(ch == NCH - 1))

                run(stageA1t, t - 1)
                run(stageC, t - 4, quarters=False)
                run(stageD, t - 5, quarters=False)

    nc.compile()
    return nc


# ---------------------------------------------------------------- host prep
def _prep_core(c, w1, b1, w2, b2, w3, b3, in1):
    """Build the per-core input arrays (index/layout placement only)."""
    bf16 = _np_bf16()
    fp8 = _np_fp8()
    MC = GC * W  # 10000 real nodes per core

    # --- layer-1 dense matrix [1024, MP], columns = local node id 4g+j
    m_glob0 = (GC * c) * W
    e_idx = m_glob0 * FANIN + np.arange(MC * FANIN)
    t = in1[e_idx].astype(np.int64)                 # [MC*16]
    wv = w1[e_idx].astype(np.float64)
    mloc = np.repeat(np.arange(MC, dtype=np.int64), FANIN)
    A1 = np.bincount(t * MP + mloc, weights=wv,
                     minlength=N_TF * MP).reshape(N_TF, MP)
    # flat layout: a1[p, tt*4096 + ch*512 + j] = A1s[ch*128+p, tt*512+j]
    a1_packed = ((A1 * A1SCALE).reshape(NCH, 128, NT, MT)
                 .transpose(1, 2, 0, 3)
                 .reshape(128, NT * NCH * MT)
                 .astype(np.float32).astype(fp8))

    b1p = np.zeros(MP, np.float32)
    b1p[:MC] = b1[m_glob0:m_glob0 + MC]
    b1c = np.ascontiguousarray(b1p.reshape(NPT, 128).T).astype(bf16)

    # --- padded per-gene weights
    w2pad = np.zeros((GP, W, W), np.float32)        # [gene, i, j]
    w2pad[:GC] = w2.reshape(N_GENES, W, W)[GC * c:GC * (c + 1)]
    b2pad = np.zeros((GP, W), np.float32)
    b2pad[:GC] = b2.reshape(N_GENES, W)[GC * c:GC * (c + 1)]
    w3pad = np.zeros((GP, W), np.float32)
    w3pad[:GC] = w3.reshape(N_GENES, W)[GC * c:GC * (c + 1)]
    b3pad = np.zeros(GP, np.float32)
    b3pad[:GC] = b3[GC * c:GC * (c + 1)]

    # --- compact W2: w2c[tt, p=(a,j), q*4+i] = w2[(tt*4+q)*32+a, i, j];
    # the kernel expands it to the block-diag stationary on device
    w2c = (w2pad.reshape(NT, 4, 32, W, W)
           .transpose(0, 2, 4, 1, 3)       # [tt, a, j, q, i]
           .reshape(NT, 128, 16))

    # --- W3 mats: W3m[pt, (a,i), b] = d(a==b) w3[g*4+i]
    idx = np.arange(32)
    W3m = np.zeros((NPT, 32, W, 32), np.float32)
    W3m[:, idx, :, idx] = w3pad.reshape(NPT, 32, W).transpose(1, 0, 2)
    w3m = W3m.reshape(NT, 4, 128, 32)   # [tt, q, (a,j), b]

    # fused per-tile weight stream: 16 compact w2 cols + 128 w3 cols
    w23 = np.zeros((NT, 128, 144), np.float32)
    w23[:, :, :16] = w2c
    w23[:, :, 16:] = w3m.transpose(0, 2, 1, 3).reshape(NT, 128, 128)
    w23m = np.ascontiguousarray(
        w23.transpose(1, 0, 2).reshape(128, NT * 144)).astype(bf16)

    # --- bias columns
    b2c = (b2pad.reshape(NPT, 32, W).transpose(1, 2, 0)
           .reshape(128, NPT).astype(np.float32))
    b3c = np.ascontiguousarray(b3pad.reshape(NT, 128).T)

    return {
        "a1": a1_packed,
        "b1c": b1c,
        "w23m": w23m,
        "b2c": b2c,
        "b3c": b3c,
    }


def _run(inputs, trace=False):
    global _COMPILED
    if _COMPILED is None:
        _COMPILED = _build_program()
    nc = _COMPILED

    bf16 = _np_bf16()
    features = np.asarray(inputs["features"], np.float32)
    w1 = np.asarray(inputs["w1"], np.float32)
    b1 = np.asarray(inputs["b1"], np.float32)
    w2 = np.asarray(inputs["w2"], np.float32)
    b2 = np.asarray(inputs["b2"], np.float32)
    w3 = np.asarray(inputs["w3"], np.float32)
    b3 = np.asarray(inputs["b3"], np.float32)
    in1 = np.asarray(inputs["in1"], np.int32)

    # stationary x: [p, ch*128 + b] = x[b, ch*128 + p]
    xt = (features.T.reshape(NCH, 128, B).transpose(1, 0, 2)
          .reshape(128, N_TF).astype(bf16))
    ident = np.eye(128, dtype=np.float32).astype(bf16)
    maskb = np.kron(np.eye(32, dtype=np.float32),
                    np.ones((4, 4), np.float32)).astype(bf16)

    in_maps = []
    for c in range(NCORES):
        m = _prep_core(c, w1, b1, w2, b2, w3, b3, in1)
        m["xt"] = xt
        m["ident"] = ident
        m["maskb"] = maskb
        in_maps.append(m)

    if trace:
        _install_ntff_shim()
    res = run_bass_kernel_spmd(nc, in_maps, core_ids=list(range(NCORES)),
                               trace=trace)
    y = np.empty((B, N_GENES), np.float32)
    for c in range(NCORES):
        yc = np.asarray(res.results[c]["out"]).astype(np.float32)  # [128,NT*128]
        # yc[p, tt*128 + b] = y[b, GC*c + tt*128 + p]
        yg = yc.reshape(128, NT, 128).transpose(2, 1, 0).reshape(B, GP)
        y[:, GC * c:GC * (c + 1)] = yg[:, :GC]
    return y, res.exec_time_ns


def kernel(**inputs) -> np.ndarray:
    y, _ = _run(inputs, trace=False)
    return y


# revision 61
# speedup vs baseline: 1.0410x; 1.0105x over previous
"""Trainium2 Bass kernel for the gene-network AE decoder (3 sparse layers).

Network (per reference):
  h1 = tanh(x @ A1 + b1)                A1: [1024, 80000], 16 nnz/col
  h2 = tanh(blockdiag4x4(W2) h1 + b2)   gene-local 4x4 mixing
  y  = blockdiag1x4(W3) h2 + b3         gene-local 4->1 reduction

Sharding: genes across the 8 cores (2500 genes -> padded to 2560 = 10240
nodes = 20 matmul tiles of 512). No inter-core communication: layer 1 only
needs the (replicated) 1024 TF features; layers 2/3 are gene-local.

The layer-1 sparse matrix is expanded to dense fp8 e3m4 on the host
(placement of the runtime w1 values at positions given by the runtime in1
indices; all arithmetic happens on device). e3m4 halves the dominant HBM
stream; w2/w3 stay bf16 (total quantization error ~1.5%, gate 2%).

Pipeline (per m-tile of 512 nodes = 128 genes), software-pipelined with
one-iteration slack between engine stages:

  t=tt:   L1   (PE)  8 chunk matmuls, xt stationary / a1 moving -> ps1
  t=tt+1: EVAC (ACT) ps1/8 -> s1 bf16;  T (PE, after L1(t)) -> psT [m,b];
          W2-expand (Pool) compact 16-col w2 -> 512-col block-diag
  t=tt+2: ADD1 (DVE) +b1;  TANH1 (ACT) -> h1T
  t=tt+3: L2   (PE)  4 block-diag W2 matmuls; ADD2 (DVE) +b2;
          TANH2 (ACT) -> h2T
  t=tt+4: L3   (PE)  4 W3 matmuls packed in one PSUM tile; ADD3 (DVE) +b3
          into a 2-tile output buffer
  t=tt+5: out DMA for each tile pair (odd tt)

PE is the binding engine (L1 37us + T 7 + L2 6 + L3 4 busy, ~96%
occupancy in span). Hard-won scheduling facts baked in here:
- each dma_start costs ~0.65us of sequencer issue time (DIRECT2D), so a1
  ships as 2-tile (1MB) DMAs from a flat [128, NT*4096] layout, w2/w3
  ride one fused 144-col stream (w2 ships compact and is expanded into
  its 97%-zeros block-diagonal form by the otherwise-idle Pool engine:
  mask x broadcast multiply), and outputs pair up 2 tiles per DMA;
- the sync HW queue starts ~2.4us before the scalar one, so xt leads the
  sync queue ahead of the a1 stream;
- a DMA whose semaphore wait isn't already satisfied blocks its queue's
  sequencer in-order, so the out DMA fires one iteration AFTER its data
  is complete, and the sync queue carries nothing but the a1 stream;
- the DMA XBAR transpose (dma_start_transpose) is NOT free - it occupies
  the issuing engine ~1.1us per [128,512] - so transposes stay on PE;
- fp8 e3m4 runs at 1 cycle/row (same as bf16) on PE; DoubleRow (0.5
  cyc/row) requires e4m3 whose 3 mantissa bits measure 2.8-4.0% rel err
  end-to-end - over the 2% gate, so no DoubleRow;
- a dummy tanh right after the const DMAs preloads the 1.3us ACT table
  during the DMA ramp;
- ptile q3 of the last tile is pure padding: its A/B chain is skipped,
  L1(last) runs 384-wide, and L3 uses a zeroed stationary stand-in.
Note: some runs execute at a 1.2x slower DVFS point (ACT_TABLE_LOAD
canary 1539ns vs nominal 1283ns); nominal-clock time is ~73-75us.
"""

import sys
import types

import numpy as np

try:
    import ml_dtypes
except ImportError:  # pragma: no cover
    ml_dtypes = None

import concourse.bass as bass
import concourse.tile as tile
from concourse import bacc, mybir
from concourse.bass_utils import run_bass_kernel_spmd

# ---------------------------------------------------------------- constants
B = 128          # batch
N_TF = 1024      # input features (= 8 chunks of 128)
N_GENES = 20000
W = 4            # nodes per gene
FANIN = 16
NCORES = 8
GC = N_GENES // NCORES      # 2500 genes / core
GP = 2560                   # padded genes / core
MP = GP * W                 # 10240 padded nodes / core
MT = 512                    # matmul moving tile (1 PSUM bank of f32)
NT = MP // MT               # 20 tiles / core
NCH = N_TF // 128           # 8 contraction chunks
A1SCALE = 8.0    # fp8e3 pre-scale: keeps w1 out of the e3m4 subnormal range
NPT = MP // 128             # 80 ptiles (128 nodes = 32 genes)

BF16 = mybir.dt.bfloat16
F32 = mybir.dt.float32
FP8 = mybir.dt.float8e3

_COMPILED = None


def _np_bf16():
    assert ml_dtypes is not None, "ml_dtypes required for bf16 host arrays"
    return ml_dtypes.bfloat16


def _np_fp8():
    assert ml_dtypes is not None, "ml_dtypes required for fp8 host arrays"
    return ml_dtypes.float8_e3m4


# ---------------------------------------------------------------- NTFF shim
def _install_ntff_shim():
    """Register the NTFF profile hook if this image's antenv lacks it."""
    try:
        import antenv
        if "antenv.axon_hooks" in sys.modules:
            return
        mod = types.ModuleType("antenv.axon_hooks")
        mod._hook = None
        mod.set_axon_ntff_profile_hook = lambda h: setattr(mod, "_hook", h)
        mod.get_axon_ntff_profile_hook = lambda: mod._hook
        sys.modules["antenv.axon_hooks"] = mod
        antenv.axon_hooks = mod
        from trn_agent_boot.trn_boot import _ntff_profile_via_ctypes
        mod.set_axon_ntff_profile_hook(
            _ntff_profile_via_ctypes("/opt/axon/libaxon_pjrt.so"))
    except Exception:
        pass


# ---------------------------------------------------------------- program
def _build_program():
    nc = bacc.Bacc("TRN2", target_bir_lowering=False, debug=False,
                   num_devices=NCORES)

    a1_ext = nc.dram_tensor("a1", [128, NT * NCH * MT], FP8,
                            kind="ExternalInput")
    xt_ext = nc.dram_tensor("xt", [128, N_TF], BF16, kind="ExternalInput")
    b1_ext = nc.dram_tensor("b1c", [128, NPT], BF16, kind="ExternalInput")
    w23_ext = nc.dram_tensor("w23m", [128, NT * 144], BF16,
                             kind="ExternalInput")
    mask_ext = nc.dram_tensor("maskb", [128, 128], BF16,
                              kind="ExternalInput")
    b2_ext = nc.dram_tensor("b2c", [128, NPT], F32, kind="ExternalInput")
    b3_ext = nc.dram_tensor("b3c", [128, NT], F32, kind="ExternalInput")
    id_ext = nc.dram_tensor("ident", [128, 128], BF16, kind="ExternalInput")
    # out[p, tt*128 + b] = y[b, gene tt*128 + p] (tile-major columns)
    out_ext = nc.dram_tensor("out", [128, NT * 128], BF16,
                             kind="ExternalOutput")

    LAST = NT - 1

    with tile.TileContext(nc) as tc:
        with (
            tc.tile_pool(name="consts", bufs=1) as consts,
            tc.tile_pool(name="a1p", bufs=4) as a1p,
            tc.tile_pool(name="w2sp", bufs=7) as w2sp,
            tc.tile_pool(name="w2xp", bufs=4) as w2xp,
            tc.tile_pool(name="ps1p", bufs=2, space="PSUM") as ps1p,
            tc.tile_pool(name="ps2p", bufs=2, space="PSUM") as ps2p,
            tc.tile_pool(name="ps3p", bufs=2, space="PSUM") as ps3p,
            tc.tile_pool(name="s1p", bufs=3) as s1p,
            tc.tile_pool(name="s1Tp", bufs=2, space="PSUM") as s1Tp,
            tc.tile_pool(name="s2p", bufs=3) as s2p,
            tc.tile_pool(name="s3p", bufs=3) as s3p,
            tc.tile_pool(name="h1Tp", bufs=3) as h1Tp,
            tc.tile_pool(name="h2Tp", bufs=3) as h2Tp,
            tc.tile_pool(name="outp", bufs=3) as outp,
        ):
            # the sync HW queue starts executing ~2.4us before the scalar
            # one, so xt leads the sync queue ahead of the a1 stream (both
            # gate the first matmul); consts ride the late scalar queue,
            # ident first (T(0) needs it soonest)
            # first 2 xt chunks ahead of a1 chunks 0-1 so L1(0) starts
            # after only 192KB; the rest streams behind the first matmuls
            xt = consts.tile([128, N_TF], BF16, tag="xt")
            nc.sync.dma_start(xt[:, :256], xt_ext.ap()[:, :256])
            ident = consts.tile([128, 128], BF16, tag="ident")
            nc.scalar.dma_start(ident[:], id_ext.ap())
            b1c = consts.tile([128, NPT], BF16, tag="b1c")
            nc.scalar.dma_start(b1c[:], b1_ext.ap())
            b2c = consts.tile([128, NPT], F32, tag="b2c")
            nc.scalar.dma_start(b2c[:], b2_ext.ap())
            b3c = consts.tile([128, NT], F32, tag="b3c")
            nc.scalar.dma_start(b3c[:], b3_ext.ap())
            # 4x4 block-diagonal ones mask for the on-device w2 expansion
            maskb = consts.tile([128, 128], BF16, tag="maskb")
            nc.scalar.dma_start(maskb[:], mask_ext.ap())
            # stand-in for the all-padding ptile q3 of the last tile
            zero_h2 = consts.tile([128, 128], BF16, tag="zero_h2")
            nc.gpsimd.memset(zero_h2[:], 0)
            # preload the tanh ACT table during the DMA ramp so the first
            # real tanh doesn't eat the 1.3us table switch (reads xt: it
            # lands early on the sync queue)
            warm = consts.tile([128, 1], BF16, tag="warm")
            nc.scalar.activation(warm[:], xt[:, :1],
                                 mybir.ActivationFunctionType.Tanh)

            st = {}   # tile index -> dict of live tensors

            def stageW(tt):
                """expand compact w2 (16 cols) into the 512-col block-diag
                stationary on the idle Pool engine: 1 iter after its DMA."""
                d = st.setdefault(tt, {})
                w23t = d["w23t"]
                w2x = w2xp.tile([128, 512], BF16, tag="w2x",
                                name=f"w2x_{tt}")
                nc.gpsimd.tensor_tensor(
                    w2x[:].rearrange("p (q b i) -> p q b i", q=4, b=32),
                    maskb[:].rearrange("p (b i) -> p b i", b=32)[
                        :, None, :, :].to_broadcast([128, 4, 32, 4]),
                    w23t[:, :16].rearrange("p (q i) -> p q i", q=4)[
                        :, :, None, :].to_broadcast([128, 4, 32, 4]),
                    mybir.AluOpType.mult)
                d["w2x"] = w2x

            def stageA1e(tt, q0=0, q1=4):
                """evac (ACT): 1 iter after L1."""
                d = st[tt]
                nq = q1 - q0
                sfx = f"{tt}_{q0}"
                s1 = s1p.tile([128, nq * 128], BF16, tag="s1",
                              name=f"s1_{sfx}")
                nc.scalar.activation(s1[:], d["ps1"][:, q0 * 128:q1 * 128],
                                     mybir.ActivationFunctionType.Copy,
                                     scale=1.0 / A1SCALE)
                d[f"s1_{q0}"] = s1

            def stageA1t(tt, q0=0, q1=4):
                """transpose (PE): 1 iter after L1, placed after L1(t) in
                the PE stream so the evac has most of an iteration of
                slack."""
                d = st[tt]
                nq = q1 - q0
                sfx = f"{tt}_{q0}"
                s1 = d.pop(f"s1_{q0}")
                psT = s1Tp.tile([128, nq, 128], BF16, tag="psT",
                                name=f"psT_{sfx}")
                for q in range(nq):
                    nc.tensor.transpose(psT[:, q, :],
                                        s1[:, q * 128:(q + 1) * 128],
                                        ident[:])
                d[f"s1T_{q0}"] = psT

            def stageA2(tt, q0=0, q1=4):
                """+b1 (DVE, broadcast over batch) + tanh (ACT): 2 iters
                after L1."""
                d = st[tt]
                nq = q1 - q0
                sfx = f"{tt}_{q0}"
                s1T = d.pop(f"s1T_{q0}")
                s2 = s2p.tile([128, nq * 128], BF16, tag="s2",
                              name=f"s2_{sfx}")
                nc.vector.tensor_tensor(
                    s2[:].rearrange("p (q b) -> p q b", q=nq),
                    s1T[:],
                    b1c[:, tt * 4 + q0:tt * 4 + q1, None].to_broadcast(
                        [128, nq, 128]),
                    mybir.AluOpType.add)
                h1T = h1Tp.tile([128, nq * 128], BF16, tag="h1T",
                                name=f"h1T_{sfx}")
                nc.scalar.activation(h1T[:], s2[:],
                                     mybir.ActivationFunctionType.Tanh)
                d[f"h1T_{q0}"] = h1T

            def stageB(tt, q0=0, q1=4):
                """layer 2 (PE) + bias (DVE) + tanh (ACT): 3 iters after
                L1."""
                d = st[tt]
                nq = q1 - q0
                sfx = f"{tt}_{q0}"
                h1T = d.pop(f"h1T_{q0}")
                ps2 = ps2p.tile([128, nq * 128], F32, tag="ps2",
                                name=f"ps2_{sfx}")
                for q in range(nq):
                    nc.tensor.matmul(
                        ps2[:, q * 128:(q + 1) * 128],
                        d["w2x"][:, (q0 + q) * 128:(q0 + q + 1) * 128],
                        h1T[:, q * 128:(q + 1) * 128],
                        start=True, stop=True)
                s3 = s3p.tile([128, nq * 128], F32, tag="s3",
                              name=f"s3_{sfx}")
                nc.vector.tensor_tensor(
                    s3[:].rearrange("p (q b) -> p q b", q=nq),
                    ps2[:].rearrange("p (q b) -> p q b", q=nq),
                    b2c[:, tt * 4 + q0:tt * 4 + q1, None].to_broadcast(
                        [128, nq, 128]),
                    mybir.AluOpType.add)
                h2T = h2Tp.tile([128, nq * 128], BF16, tag="h2T",
                                name=f"h2T_{sfx}")
                nc.scalar.activation(h2T[:], s3[:],
                                     mybir.ActivationFunctionType.Tanh)
                d[f"h2T_{q0}"] = h2T

            def stageC(tt):
                """layer 3 (PE, full width) + b3 (DVE): 4 iters after L1."""
                d = st[tt]
                ps3 = ps3p.tile([128, 128], F32, tag="ps3",
                                name=f"ps3_{tt}")
                for q in range(4):
                    if tt == LAST:
                        # ptile q3 of the last tile is all padding
                        h2q = (zero_h2[:] if q == 3
                               else d.pop(f"h2T_{q}")[:, :128])
                    elif q == 0:
                        d["_h2T"] = d.pop("h2T_0")
                        h2q = d["_h2T"][:, :128]
                    else:
                        h2q = d["_h2T"][:, q * 128:(q + 1) * 128]
                    nc.tensor.matmul(
                        ps3[q * 32:(q + 1) * 32, :],
                        d["w3t"][:, q * 32:(q + 1) * 32],
                        h2q,
                        start=True, stop=True,
                        tile_position=(0, 32 * q))
                d.pop("_h2T", None)
                if tt % 2 == 0:
                    yt = outp.tile([128, 256], BF16, tag="yt",
                                   name=f"yt_{tt // 2}")
                    st[tt + 1]["ytbuf"] = yt
                else:
                    yt = d.pop("ytbuf")
                nc.vector.tensor_scalar_add(
                    yt[:, (tt % 2) * 128:(tt % 2 + 1) * 128], ps3[:],
                    b3c[:128, tt:tt + 1])
                d["yt"] = yt

            def stageD(tt):
                """out DMA for tile pair (tt-1, tt), odd tt: 5 iters after
                L1, one iteration after its yt half is written so the sem
                wait never blocks the queue sequencer."""
                if tt % 2 == 0:
                    return
                yt = st[tt].pop("yt")
                nc.scalar.dma_start(
                    out_ext.ap()[:, (tt - 1) * 128:(tt + 1) * 128], yt[:])

            def run(stage, tl, quarters=True):
                if 0 <= tl < NT:
                    if tl == LAST and quarters:
                        # q3 is all padding: skip its whole A/B chain
                        for q in range(3):
                            stage(tl, q, q + 1)
                    else:
                        stage(tl)

            for t in range(NT + 5):
                # ---- input DMAs (each dma_start costs ~0.65us of
                # sequencer issue time, so a1 rides in 2-tile chunks and
                # w2/w3 are fused into one 640-col stream)
                TW = NCH * MT   # 4096 a1 cols per m-tile
                if t < NT and t % 2 == 0:
                    a1t = a1p.tile([128, 2 * TW], FP8,
                                   tag="a1t", name=f"a1t_{t // 2}")
                    if t == 0:
                        # chunks 0-1 first so L1(0) starts after 128KB,
                        # then the rest of xt, then a1 chunks 2-7, then
                        # tile 1 in halves so L1(1) starts early too
                        nc.sync.dma_start(a1t[:, :2 * MT],
                                          a1_ext.ap()[:, :2 * MT])
                        nc.sync.dma_start(xt[:, 256:], xt_ext.ap()[:, 256:])
                        nc.sync.dma_start(a1t[:, 2 * MT:TW],
                                          a1_ext.ap()[:, 2 * MT:TW])
                        nc.sync.dma_start(a1t[:, TW:TW + TW // 2],
                                          a1_ext.ap()[:, TW:TW + TW // 2])
                        nc.sync.dma_start(a1t[:, TW + TW // 2:],
                                          a1_ext.ap()[:, TW + TW // 2:2 * TW])
                    elif t in (2, 4):
                        # single-tile arrival granularity while the DMA
                        # stream is still catching up to PE demand
                        nc.sync.dma_start(a1t[:, :TW],
                                          a1_ext.ap()[:, t * TW:(t + 1) * TW])
                        nc.sync.dma_start(a1t[:, TW:],
                                          a1_ext.ap()[:, (t + 1) * TW:(t + 2) * TW])
                    else:
                        nc.sync.dma_start(
                            a1t[:], a1_ext.ap()[:, t * TW:(t + 2) * TW])
                    st.setdefault(t, {})["a1t"] = a1t
                    st.setdefault(t + 1, {})["a1t"] = a1t
                if t < NT:
                    # compact stream: 16 w2 cols (4x4 per gene-node) + 128
                    # dense w3 cols; w2 is expanded on the idle Pool engine
                    w23t = w2sp.tile([128, 144], BF16, tag="w23t",
                                     name=f"w23t_{t}")
                    nc.scalar.dma_start(
                        w23t[:], w23_ext.ap()[:, t * 144:(t + 1) * 144])

                # engine-stream order per iteration:
                #   ACT: evac(t-1), tanh1(t-2), tanh2(t-3)
                #   DVE: add1(t-2), add2(t-3), add3(t-4)
                #   PE:  L2(t-3), L1(t), T(t-1), L3(t-4) — L2 before L1 so
                #        PE does ready work while a1(t) may still be in
                #        flight (L1-first measured 2us slower)
                run(stageA1e, t - 1)
                run(stageW, t - 1, quarters=False)
                run(stageA2, t - 2)
                run(stageB, t - 3)

                if t < NT:
                    ps1 = ps1p.tile([128, MT], F32, tag="ps1",
                                    name=f"ps1_{t}")
                    a1v = st[t]
                    a1v["ps1"] = ps1
                    a1v["w23t"] = w23t[:]
                    a1v["w3t"] = w23t[:, 16:144]
                    a1m = a1v.pop("a1t")
                    off = (t % 2) * TW
                    mw = 384 if t == LAST else MT  # last ptile is padding
                    for ch in range(NCH):
                        nc.tensor.matmul(
                            ps1[:, :mw],
                            xt[:, ch * 128:(ch + 1) * 128],
                            a1m[:, off + ch * MT:off + ch * MT + mw],
                            start=(ch == 0), stop=(ch == NCH - 1))

                run(stageA1t, t - 1)
                run(stageC, t - 4, quarters=False)
                run(stageD, t - 5, quarters=False)

    nc.compile()
    return nc


# ---------------------------------------------------------------- host prep
def _prep_core(c, w1, b1, w2, b2, w3, b3, in1):
    """Build the per-core input arrays (index/layout placement only)."""
    bf16 = _np_bf16()
    fp8 = _np_fp8()
    MC = GC * W  # 10000 real nodes per core

    # --- layer-1 dense matrix [1024, MP], columns = local node id 4g+j
    m_glob0 = (GC * c) * W
    e_idx = m_glob0 * FANIN + np.arange(MC * FANIN)
    t = in1[e_idx].astype(np.int64)                 # [MC*16]
    wv = w1[e_idx].astype(np.float64)
    mloc = np.repeat(np.arange(MC, dtype=np.int64), FANIN)
    A1 = np.bincount(t * MP + mloc, weights=wv,
                     minlength=N_TF * MP).reshape(N_TF, MP)
    # flat layout: a1[p, tt*4096 + ch*512 + j] = A1s[ch*128+p, tt*512+j]
    a1_packed = ((A1 * A1SCALE).reshape(NCH, 128, NT, MT)
                 .transpose(1, 2, 0, 3)
                 .reshape(128, NT * NCH * MT)
                 .astype(np.float32).astype(fp8))

    b1p = np.zeros(MP, np.float32)
    b1p[:MC] = b1[m_glob0:m_glob0 + MC]
    b1c = np.ascontiguousarray(b1p.reshape(NPT, 128).T).astype(bf16)

    # --- padded per-gene weights
    w2pad = np.zeros((GP, W, W), np.float32)        # [gene, i, j]
    w2pad[:GC] = w2.reshape(N_GENES, W, W)[GC * c:GC * (c + 1)]
    b2pad = np.zeros((GP, W), np.float32)
    b2pad[:GC] = b2.reshape(N_GENES, W)[GC * c:GC * (c + 1)]
    w3pad = np.zeros((GP, W), np.float32)
    w3pad[:GC] = w3.reshape(N_GENES, W)[GC * c:GC * (c + 1)]
    b3pad = np.zeros(GP, np.float32)
    b3pad[:GC] = b3[GC * c:GC * (c + 1)]

    # --- compact W2: w2c[tt, p=(a,j), q*4+i] = w2[(tt*4+q)*32+a, i, j];
    # the kernel expands it to the block-diag stationary on device
    w2c = (w2pad.reshape(NT, 4, 32, W, W)
           .transpose(0, 2, 4, 1, 3)       # [tt, a, j, q, i]
           .reshape(NT, 128, 16))

    # --- W3 mats: W3m[pt, (a,i), b] = d(a==b) w3[g*4+i]
    idx = np.arange(32)
    W3m = np.zeros((NPT, 32, W, 32), np.float32)
    W3m[:, idx, :, idx] = w3pad.reshape(NPT, 32, W).transpose(1, 0, 2)
    w3m = W3m.reshape(NT, 4, 128, 32)   # [tt, q, (a,j), b]

    # fused per-tile weight stream: 16 compact w2 cols + 128 w3 cols
    w23 = np.zeros((NT, 128, 144), np.float32)
    w23[:, :, :16] = w2c
    w23[:, :, 16:] = w3m.transpose(0, 2, 1, 3).reshape(NT, 128, 128)
    w23m = np.ascontiguousarray(
        w23.transpose(1, 0, 2).reshape(128, NT * 144)).astype(bf16)

    # --- bias columns
    b2c = (b2pad.reshape(NPT, 32, W).transpose(1, 2, 0)
           .reshape(128, NPT).astype(np.float32))
    b3c = np.ascontiguousarray(b3pad.reshape(NT, 128).T)

    return {
        "a1": a1_packed,
        "b1c": b1c,
        "w23m": w23m,
        "b2c": b2c,
        "b3c": b3c,
    }


def _run(inputs, trace=False):
    global _COMPILED
    if _COMPILED is None:
        _COMPILED = _build_program()
    nc = _COMPILED

    bf16 = _np_bf16()
    features = np.asarray(inputs["features"], np.float32)
    w1 = np.asarray(inputs["w1"], np.float32)
    b1 = np.asarray(inputs["b1"], np.float32)
    w2 = np.asarray(inputs["w2"], np.float32)
    b2 = np.asarray(inputs["b2"], np.float32)
    w3 = np.asarray(inputs["w3"], np.float32)
    b3 = np.asarray(inputs["b3"], np.float32)
    in1 = np.asarray(inputs["in1"], np.int32)

    # stationary x: [p, ch*128 + b] = x[b, ch*128 + p]
    xt = (features.T.reshape(NCH, 128, B).transpose(1, 0, 2)
          .reshape(128, N_TF).astype(bf16))
    ident = np.eye(128, dtype=np.float32).astype(bf16)
    maskb = np.kron(np.eye(32, dtype=np.float32),
                    np.ones((4, 4), np.float32)).astype(bf16)

    in_maps = []
    for c in range(NCORES):
        m = _prep_core(c, w1, b1, w2, b2, w3, b3, in1)
        m["xt"] = xt
        m["ident"] = ident
        m["maskb"] = maskb
        in_maps.append(m)

    if trace:
        _install_ntff_shim()
    res = run_bass_kernel_spmd(nc, in_maps, core_ids=list(range(NCORES)),
                               trace=trace)
    y = np.empty((B, N_GENES), np.float32)
    for c in range(NCORES):
        yc = np.asarray(res.results[c]["out"]).astype(np.float32)  # [128,NT*128]
        # yc[p, tt*128 + b] = y[b, GC*c + tt*128 + p]
        yg = yc.reshape(128, NT, 128).transpose(2, 1, 0).reshape(B, GP)
        y[:, GC * c:GC * (c + 1)] = yg[:, :GC]
    return y, res.exec_time_ns


def kernel(**inputs) -> np.ndarray:
    y, _ = _run(inputs, trace=False)
    return y
